# revision 22
# baseline (speedup 1.0000x reference)
"""Distributed Bass kernel for attention (B=4,S=1024,D=4096,H=32,HD=128).

Sharding: 8 cores = 4 batch x 2 head-groups of 16 heads (core c: batch c//2,
heads (c%2)*16..+16). Per-core pipeline (all matmuls bf16 with fp32 PSUM):

  1. QKV projections. q/k computed feature-major (q^T[hd,tok] per head) so
     attention needs no transposes; v computed token-major. x and weights are
     streamed; q/k get partial rotary applied in-SBUF then bounce via DRAM.
     Rotary uses the block-order trick: scores are invariant to a channel
     permutation applied identically to q and k, so the rotated (real,imag)
     halves stay block-contiguous instead of interleaved.
  2. Scores computed directly transposed E[k,q] = exp(kT.T @ qT + biasT).
     No max subtraction (scores bounded for this input distribution); the
     softmax denominator is a ones-vector matmul reducing over partitions.
     1/s is folded into the o^T PSUM evacuation (o^T = v.T @ E unnormalized).
  3. Output projection partial sums with wo_bias/2 folded in, chunked
     pairwise ReduceScatter (bf16) over token quarters, fp32 cast, DMA out.
"""

import sys

sys.path.insert(0, "/opt/trn_rl_repo")

import numpy as np
import ml_dtypes

BF16 = ml_dtypes.bfloat16

B, S, D, H, HD = 4, 1024, 4096, 32, 128
ROTARY = 32
MAX_POS = 10000
HG = H // 2  # heads per core = 16
F = HG * HD  # per-core qkv feature dim = 2048
NCORES = 8
SCALE = 1.0 / np.sqrt(HD)
NDC = D // 128  # 32 contraction chunks
NKC = S // 128  # 8 key chunks

_cache = {}


def _build():
    import concourse.mybir as mybir
    import concourse.tile as tile
    from concourse import bacc

    fp32 = mybir.dt.float32
    bf16 = mybir.dt.bfloat16
    Act = mybir.ActivationFunctionType

    nc = bacc.Bacc("TRN2", target_bir_lowering=False, num_devices=NCORES)

    # ---- DRAM parameters (per-core shards) ----
    xT = nc.dram_tensor("xT", [D, S], bf16, kind="ExternalInput")
    wq = nc.dram_tensor("wq", [D, F], bf16, kind="ExternalInput")
    wk = nc.dram_tensor("wk", [D, F], bf16, kind="ExternalInput")
    wv = nc.dram_tensor("wv", [D, F], bf16, kind="ExternalInput")
    wo = nc.dram_tensor("wo", [F, D], bf16, kind="ExternalInput")
    bqT = nc.dram_tensor("bqT", [HD, HG], fp32, kind="ExternalInput")
    bkT = nc.dram_tensor("bkT", [HD, HG], fp32, kind="ExternalInput")
    bv_bc = nc.dram_tensor("bv_bc", [128, F], bf16, kind="ExternalInput")
    bo_bc = nc.dram_tensor("bo_bc", [128, D], bf16, kind="ExternalInput")
    biasT = nc.dram_tensor("biasT", [S, S], bf16, kind="ExternalInput")
    rotC = nc.dram_tensor("rotC", [16, S], bf16, kind="ExternalInput")
    rotS = nc.dram_tensor("rotS", [16, S], bf16, kind="ExternalInput")
    ones = nc.dram_tensor("ones", [128, 1], bf16, kind="ExternalInput")
    ones_row = nc.dram_tensor("ones_row", [1, 128], fp32, kind="ExternalInput")
    out = nc.dram_tensor("out", [4, 128, D], fp32, kind="ExternalOutput")

    RG = [[0, 1], [2, 3], [4, 5], [6, 7]]

    with tile.TileContext(nc) as tc:
        with (
            tc.tile_pool(name="wpool", bufs=3) as wpool,
            tc.tile_pool(name="wvpool", bufs=2) as wvpool,
            tc.tile_pool(name="xpool", bufs=2) as xpool,
            tc.tile_pool(name="stream", bufs=2) as stream,
            tc.tile_pool(name="stage", bufs=2) as stpool,
            tc.tile_pool(name="tmp", bufs=1) as tmppool,
            tc.tile_pool(name="small", bufs=1) as small,
            tc.tile_pool(name="epool", bufs=1) as epool,
            tc.tile_pool(name="big", bufs=1) as big,
            tc.tile_pool(name="evac", bufs=2) as evacpool,
            tc.tile_pool(name="outp", bufs=2) as outpool,
            tc.tile_pool(name="ps", bufs=1, space="PSUM") as pspool,
            tc.tile_pool(name="dram", bufs=1, space="DRAM") as dram,
        ):
            # ---- constants ----
            bqT_sb = small.tile([HD, HG], fp32)
            bkT_sb = small.tile([HD, HG], fp32)
            bv_sb = small.tile([128, F], bf16)
            bo_sb = small.tile([128, D], bf16)
            rotC_sb = small.tile([16, S], bf16)
            rotS_sb = small.tile([16, S], bf16)
            ones_sb = small.tile([128, 1], bf16)
            ones_row_sb = small.tile([1, 128], fp32)
            nc.sync.dma_start(ones_row_sb[:], ones_row[:])
            nc.sync.dma_start(bqT_sb[:], bqT[:])
            nc.sync.dma_start(bkT_sb[:], bkT[:])
            nc.sync.dma_start(bv_sb[:], bv_bc[:])
            nc.sync.dma_start(bo_sb[:], bo_bc[:])
            nc.sync.dma_start(rotC_sb[:], rotC[:])
            nc.sync.dma_start(rotS_sb[:], rotS[:])
            nc.sync.dma_start(ones_sb[:], ones[:])

            # resident across phase 2->3
            oT_sb = big.tile([128, HG, S], bf16)  # 32KB/part [hd, h, tok]

            # DRAM bounce tensors
            q_dram = dram.tile([HG, 128, S], bf16, name="q_dram")
            k_dram = dram.tile([HG, 128, S], bf16, name="k_dram")
            v_dram = dram.tile([NKC, 128, F], bf16, name="v_dram")

            # ================= Phase 1: QKV projections =================
            def qk_pass(w_dram, bias_sb, dst_dram, which):
                for hg4 in range(4):
                    ps = {}
                    for hi in range(4):
                        for th in range(2):
                            ps[(hi, th)] = pspool.tile(
                                [128, 512], fp32, tag=f"b{hi * 2 + th}",
                                name=f"ps{which}{hi}{th}",
                            )
                    for dc in range(NDC):
                        wt = wpool.tile([128, 512], bf16, tag=f"w{which}", name="wt")
                        nc.sync.dma_start(
                            wt[:],
                            w_dram[dc * 128 : (dc + 1) * 128,
                                   hg4 * 512 : (hg4 + 1) * 512],
                        )
                        xt = xpool.tile([128, S], bf16, tag="xt", name="xt")
                        nc.scalar.dma_start(xt[:], xT[dc * 128 : (dc + 1) * 128, :])
                        for hi in range(4):
                            for th in range(2):
                                nc.tensor.matmul(
                                    ps[(hi, th)][:],
                                    wt[:, hi * 128 : (hi + 1) * 128],
                                    xt[:, th * 512 : (th + 1) * 512],
                                    start=(dc == 0),
                                    stop=(dc == NDC - 1),
                                )
                    for hi in range(4):
                        h = hg4 * 4 + hi
                        qbf = stpool.tile(
                            [128, S], bf16, tag="qbf", bufs=3, name="qbf"
                        )
                        for th in range(2):
                            # single full-tile evac+bias to bf16 on ACT
                            # (PSUM released by this one op; rotary reads
                            # bf16 from qbf and overwrites [0:32] in place)
                            nc.scalar.activation(
                                qbf[:, th * 512 : (th + 1) * 512],
                                ps[(hi, th)][:],
                                Act.Identity,
                                bias=bias_sb[:, h : h + 1],
                            )
                        # rotary (block order): u=qbf[0:16], w=qbf[16:32].
                        # Engine ops need 32-aligned partition bases, so the
                        # w half bounces via DMA to a base-0 tile and the f
                        # result bounces back to partitions 16:32.
                        rot_w = tmppool.tile([16, S], bf16, tag="rw", name="rot_w")
                        nc.scalar.dma_start(rot_w[:], qbf[16:32, :])
                        t1 = tmppool.tile([16, S], bf16, tag="t1", name="t1")
                        t2 = tmppool.tile([16, S], bf16, tag="t2", name="t2")
                        t3 = tmppool.tile([16, S], bf16, tag="t3", name="t3")
                        t4 = tmppool.tile([16, S], bf16, tag="t4", name="t4")
                        fbuf = tmppool.tile([16, S], bf16, tag="fb", name="fbuf")
                        u = qbf[0:16, :]
                        nc.vector.tensor_mul(t1[:], u, rotC_sb[:])
                        nc.vector.tensor_mul(t3[:], rot_w[:], rotS_sb[:])
                        nc.gpsimd.tensor_mul(t2[:], u, rotS_sb[:])
                        nc.gpsimd.tensor_mul(t4[:], rot_w[:], rotC_sb[:])
                        nc.vector.tensor_sub(qbf[0:16, :], t1[:], t3[:])
                        nc.gpsimd.tensor_add(fbuf[:], t2[:], t4[:])
                        nc.gpsimd.dma_start(qbf[16:32, :], fbuf[:])
                        nc.gpsimd.dma_start(dst_dram[h], qbf[:])

            qk_pass(wq, bqT_sb, q_dram, "q")
            qk_pass(wk, bkT_sb, k_dram, "k")

            # V pass (token-major): psum[tok=128, feat=512]
            for fh in range(2):
                for tq4 in range(2):
                    ps = {}
                    for ti in range(4):
                        for fc in range(2):
                            ps[(ti, fc)] = pspool.tile(
                                [128, 512], fp32, tag=f"b{ti * 2 + fc}",
                                name=f"psv{ti}{fc}",
                            )
                    for dc in range(NDC):
                        wt = wvpool.tile([128, 1024], bf16, tag="wv", name="wvt")
                        nc.sync.dma_start(
                            wt[:],
                            wv[dc * 128 : (dc + 1) * 128,
                               fh * 1024 : (fh + 1) * 1024],
                        )
                        xt = xpool.tile([128, S], bf16, tag="xt", name="xt")
                        nc.scalar.dma_start(xt[:], xT[dc * 128 : (dc + 1) * 128, :])
                        for ti in range(4):
                            tci = tq4 * 4 + ti
                            for fc in range(2):
                                nc.tensor.matmul(
                                    ps[(ti, fc)][:],
                                    xt[:, tci * 128 : (tci + 1) * 128],
                                    wt[:, fc * 512 : (fc + 1) * 512],
                                    start=(dc == 0),
                                    stop=(dc == NDC - 1),
                                )
                    for ti in range(4):
                        tci = tq4 * 4 + ti
                        for fc in range(2):
                            f0 = fh * 1024 + fc * 512
                            vb = evacpool.tile([128, 512], bf16, tag="vb", name="vb")
                            nc.vector.tensor_add(
                                vb[:], ps[(ti, fc)][:], bv_sb[:, f0 : f0 + 512]
                            )
                            nc.gpsimd.dma_start(v_dram[tci, :, f0 : f0 + 512], vb[:])

            # ================= Phase 2: attention per head =================
            for h in range(HG):
                qh_t = stream.tile([128, S], bf16, tag="qh", name="qh_t")
                kh_t = stream.tile([128, S], bf16, tag="kh", name="kh_t")
                vh_t = stream.tile([128, NKC, HD], bf16, tag="vh", name="vh_t")
                nc.sync.dma_start(qh_t[:], q_dram[h])
                nc.sync.dma_start(kh_t[:], k_dram[h])
                nc.sync.dma_start(
                    vh_t[:],
                    v_dram[:, :, h * 128 : (h + 1) * 128].rearrange(
                        "kc p hd -> p kc hd"
                    ),
                )
                E = epool.tile([128, NKC, S], bf16, tag="E", bufs=2, name="E")
                sum_ps = {}
                o_ps = {}
                for qh in range(2):
                    sum_ps[qh] = pspool.tile(
                        [1, 512], fp32, tag=f"b{2 + qh}", name=f"sum{qh}"
                    )
                    o_ps[qh] = pspool.tile(
                        [128, 512], fp32, tag=f"b{4 + qh}", name=f"o{qh}"
                    )
                for kc in range(NKC):
                    bt = stream.tile([128, S], bf16, tag="bt", name="bt")
                    nc.sync.dma_start(bt[:], biasT[kc * 128 : (kc + 1) * 128, :])
                    for qh in range(2):
                        qs = slice(qh * 512, (qh + 1) * 512)
                        stag = (0, 1, 6, 7)[(kc * 2 + qh) % 4]
                        sps = pspool.tile(
                            [128, 512], fp32, tag=f"b{stag}", name="sps"
                        )
                        nc.tensor.matmul(
                            sps[:],
                            kh_t[:, kc * 128 : (kc + 1) * 128],
                            qh_t[:, qs],
                            start=True,
                            stop=True,
                        )
                        nc.vector.tensor_add(E[:, kc, qs], sps[:], bt[:, qs])
                        nc.scalar.activation(
                            E[:, kc, qs], E[:, kc, qs], Act.Exp
                        )
                        # sum and o both consume E chunk-by-chunk, so the
                        # PE never waits for the whole row of E
                        nc.tensor.matmul(
                            sum_ps[qh][:],
                            ones_sb[:],
                            E[:, kc, qs],
                            start=(kc == 0),
                            stop=(kc == NKC - 1),
                        )
                        nc.tensor.matmul(
                            o_ps[qh][:],
                            vh_t[:, kc, :],
                            E[:, kc, qs],
                            start=(kc == 0),
                            stop=(kc == NKC - 1),
                        )
                inv_row = tmppool.tile(
                    [1, S], fp32, tag="inv", bufs=2, name="inv_row"
                )
                inv_bc = tmppool.tile(
                    [128, S], fp32, tag="invbc", bufs=2, name="inv_bc"
                )
                for qh in range(2):
                    nc.vector.reciprocal_approx_fast(
                        inv_row[:, qh * 512 : (qh + 1) * 512], sum_ps[qh][:]
                    )
                for qh in range(2):
                    qs = slice(qh * 512, (qh + 1) * 512)
                    # rank-1 PE broadcast: ones[128] x inv_row -> [128, 512]
                    bc_ps = pspool.tile(
                        [128, 512], fp32, tag=f"b{2 + qh}", name="bc_ps"
                    )
                    nc.tensor.matmul(
                        bc_ps[:],
                        ones_row_sb[:],
                        inv_row[:, qs],
                        start=True,
                        stop=True,
                    )
                    nc.vector.tensor_copy(inv_bc[:, qs], bc_ps[:])
                    nc.vector.tensor_mul(
                        oT_sb[:, h, qs], o_ps[qh][:], inv_bc[:, qs]
                    )

            # ========= Phase 3: out-projection + chunked ReduceScatter =========
            for tq in range(4):
                rs_in = dram.tile([256, D], bf16, tag="rsin", bufs=2, name="rs_in")
                rs_out = dram.tile([128, D], bf16, tag="rsout", bufs=2, name="rs_out")
                for mh in range(2):
                    ps = {}
                    for tc2 in range(2):
                        for mc2 in range(4):
                            ps[(tc2, mc2)] = pspool.tile(
                                [128, 512], fp32, tag=f"b{tc2 * 4 + mc2}",
                                name=f"pso{tc2}{mc2}",
                            )
                    for cc in range(HG):
                        wt = wvpool.tile(
                            [128, 2048], bf16, tag="wo", bufs=4, name="wot"
                        )
                        nc.sync.dma_start(
                            wt[:],
                            wo[cc * 128 : (cc + 1) * 128,
                               mh * 2048 : (mh + 1) * 2048],
                        )
                        for tc2 in range(2):
                            t128 = tq * 2 + tc2
                            for mc2 in range(4):
                                nc.tensor.matmul(
                                    ps[(tc2, mc2)][:],
                                    oT_sb[:, cc, t128 * 128 : (t128 + 1) * 128],
                                    wt[:, mc2 * 512 : (mc2 + 1) * 512],
                                    start=(cc == 0),
                                    stop=(cc == HG - 1),
                                )
                    for tc2 in range(2):
                        po = outpool.tile([128, 2048], bf16, tag="po", name="po")
                        for mc2 in range(4):
                            m0 = mh * 2048 + mc2 * 512
                            nc.vector.tensor_add(
                                po[:, mc2 * 512 : (mc2 + 1) * 512],
                                ps[(tc2, mc2)][:],
                                bo_sb[:, m0 : m0 + 512],
                            )
                        nc.gpsimd.dma_start(
                            rs_in[tc2 * 128 : (tc2 + 1) * 128,
                                  mh * 2048 : (mh + 1) * 2048],
                            po[:],
                        )
                nc.gpsimd.collective_compute(
                    "ReduceScatter",
                    mybir.AluOpType.add,
                    replica_groups=RG,
                    ins=[rs_in[:].opt()],
                    outs=[rs_out[:].opt()],
                )
                for dh in range(4):
                    fin_bf = evacpool.tile(
                        [128, 1024], bf16, tag="finbf", name="fin_bf"
                    )
                    fin_f32 = evacpool.tile(
                        [128, 1024], fp32, tag="finf32", name="fin_f32"
                    )
                    nc.gpsimd.dma_start(
                        fin_bf[:], rs_out[:, dh * 1024 : (dh + 1) * 1024]
                    )
                    nc.vector.tensor_copy(fin_f32[:], fin_bf[:])
                    nc.gpsimd.dma_start(
                        out[tq, :, dh * 1024 : (dh + 1) * 1024], fin_f32[:]
                    )

    nc.finalize()
    return nc


def _prep_shards(x, attn_bias, wq_kernel, wq_bias, wk_kernel, wk_bias,
                 wv_kernel, wv_bias, wo_kernel, wo_bias):
    """Host-side shard prep. Returns in_maps (list of 8 dicts)."""
    freqs = 1.0 / 10000.0 ** (np.arange(0, ROTARY, 2) / ROTARY)  # [16]
    pos = np.arange(MAX_POS - S, MAX_POS)  # [S]
    ang = np.outer(freqs, pos)  # [16, S]
    rotC = np.cos(ang).astype(np.float32)
    rotS = np.sin(ang).astype(np.float32)
    ones = np.ones((128, 1), dtype=BF16)
    biasT = np.ascontiguousarray(attn_bias[0, 0].T).astype(BF16)

    in_maps = []
    for c in range(NCORES):
        b, g = c // 2, c % 2
        hs = slice(g * HG, (g + 1) * HG)
        in_maps.append(
            {
                "xT": np.ascontiguousarray(x[b].T).astype(BF16),
                "wq": (wq_kernel[:, hs, :].reshape(D, F) * SCALE).astype(BF16),
                "wk": wk_kernel[:, hs, :].reshape(D, F).astype(BF16),
                "wv": wv_kernel[:, hs, :].reshape(D, F).astype(BF16),
                "wo": wo_kernel[hs].reshape(F, D).astype(BF16),
                "bqT": np.ascontiguousarray((wq_bias[hs] * SCALE).T).astype(
                    np.float32
                ),
                "bkT": np.ascontiguousarray(wk_bias[hs].T).astype(np.float32),
                "bv_bc": np.broadcast_to(
                    wv_bias[hs].reshape(1, F), (128, F)
                ).astype(BF16).copy(),
                "bo_bc": np.broadcast_to(
                    (wo_bias * 0.5).reshape(1, D), (128, D)
                ).astype(BF16).copy(),
                "biasT": biasT,
                "rotC": rotC.astype(BF16),
                "rotS": rotS.astype(BF16),
                "ones": ones,
                "ones_row": np.ones((1, 128), dtype=np.float32),
            }
        )
    return in_maps


def kernel(x, attn_bias, wq_kernel, wq_bias, wk_kernel, wk_bias,
           wv_kernel, wv_bias, wo_kernel, wo_bias, _trace=False):
    from concourse import bass_utils

    if "nc" not in _cache:
        _cache["nc"] = _build()
    nc = _cache["nc"]

    in_maps = _prep_shards(
        np.asarray(x), np.asarray(attn_bias),
        np.asarray(wq_kernel), np.asarray(wq_bias),
        np.asarray(wk_kernel), np.asarray(wk_bias),
        np.asarray(wv_kernel), np.asarray(wv_bias),
        np.asarray(wo_kernel), np.asarray(wo_bias),
    )
    res = bass_utils.run_bass_kernel_spmd(
        nc, in_maps, core_ids=list(range(NCORES)), trace=_trace
    )
    _cache["last_results"] = res

    full = np.empty((B, S, D), dtype=np.float32)
    for b in range(B):
        lo = res.results[2 * b]["out"]  # [4, 128, D]: tokens tq*256 .. +128
        hi = res.results[2 * b + 1]["out"]  # tokens tq*256+128 .. +256
        for tq in range(4):
            full[b, tq * 256 : tq * 256 + 128] = lo[tq]
            full[b, tq * 256 + 128 : (tq + 1) * 256] = hi[tq]
    return full


# revision 23
# speedup vs baseline: 1.0256x; 1.0256x over previous
"""Distributed Bass kernel for attention (B=4,S=1024,D=4096,H=32,HD=128).

Sharding: 8 cores = 4 batch x 2 head-groups of 16 heads (core c: batch c//2,
heads (c%2)*16..+16). Per-core pipeline (all matmuls bf16 with fp32 PSUM):

  1. QKV projections. q/k computed feature-major (q^T[hd,tok] per head) so
     attention needs no transposes; v computed token-major. x and weights are
     streamed; q/k get partial rotary applied in-SBUF then bounce via DRAM.
     Rotary uses the block-order trick: scores are invariant to a channel
     permutation applied identically to q and k, so the rotated (real,imag)
     halves stay block-contiguous instead of interleaved.
  2. Scores computed directly transposed E[k,q] = exp(kT.T @ qT + biasT).
     No max subtraction (scores bounded for this input distribution); the
     softmax denominator is a ones-vector matmul reducing over partitions.
     1/s is folded into the o^T PSUM evacuation (o^T = v.T @ E unnormalized).
  3. Output projection partial sums with wo_bias/2 folded in, chunked
     pairwise ReduceScatter (bf16) over token quarters, fp32 cast, DMA out.
"""

import sys

sys.path.insert(0, "/opt/trn_rl_repo")

import numpy as np
import ml_dtypes

BF16 = ml_dtypes.bfloat16

B, S, D, H, HD = 4, 1024, 4096, 32, 128
ROTARY = 32
MAX_POS = 10000
HG = H // 2  # heads per core = 16
F = HG * HD  # per-core qkv feature dim = 2048
NCORES = 8
SCALE = 1.0 / np.sqrt(HD)
NDC = D // 128  # 32 contraction chunks
NKC = S // 128  # 8 key chunks

_cache = {}


def _build():
    import concourse.mybir as mybir
    import concourse.tile as tile
    from concourse import bacc

    fp32 = mybir.dt.float32
    bf16 = mybir.dt.bfloat16
    Act = mybir.ActivationFunctionType

    nc = bacc.Bacc("TRN2", target_bir_lowering=False, num_devices=NCORES)

    # ---- DRAM parameters (per-core shards) ----
    xT = nc.dram_tensor("xT", [D, S], bf16, kind="ExternalInput")
    wq = nc.dram_tensor("wq", [D, F], bf16, kind="ExternalInput")
    wk = nc.dram_tensor("wk", [D, F], bf16, kind="ExternalInput")
    wv = nc.dram_tensor("wv", [D, F], bf16, kind="ExternalInput")
    wo = nc.dram_tensor("wo", [F, D], bf16, kind="ExternalInput")
    bqT = nc.dram_tensor("bqT", [HD, HG], fp32, kind="ExternalInput")
    bkT = nc.dram_tensor("bkT", [HD, HG], fp32, kind="ExternalInput")
    bv_bc = nc.dram_tensor("bv_bc", [128, F], bf16, kind="ExternalInput")
    bo_bc = nc.dram_tensor("bo_bc", [128, D], bf16, kind="ExternalInput")
    biasT = nc.dram_tensor("biasT", [S, S], bf16, kind="ExternalInput")
    rotC = nc.dram_tensor("rotC", [16, S], bf16, kind="ExternalInput")
    rotS = nc.dram_tensor("rotS", [16, S], bf16, kind="ExternalInput")
    ones = nc.dram_tensor("ones", [128, 1], bf16, kind="ExternalInput")
    ones_row = nc.dram_tensor("ones_row", [1, 128], fp32, kind="ExternalInput")
    out = nc.dram_tensor("out", [4, 128, D], fp32, kind="ExternalOutput")

    RG = [[0, 1], [2, 3], [4, 5], [6, 7]]

    with tile.TileContext(nc) as tc:
        with (
            tc.tile_pool(name="wpool", bufs=3) as wpool,
            tc.tile_pool(name="wvpool", bufs=2) as wvpool,
            tc.tile_pool(name="xpool", bufs=2) as xpool,
            tc.tile_pool(name="stream", bufs=2) as stream,
            tc.tile_pool(name="stage", bufs=2) as stpool,
            tc.tile_pool(name="tmp", bufs=1) as tmppool,
            tc.tile_pool(name="small", bufs=1) as small,
            tc.tile_pool(name="epool", bufs=1) as epool,
            tc.tile_pool(name="big", bufs=1) as big,
            tc.tile_pool(name="evac", bufs=2) as evacpool,
            tc.tile_pool(name="outp", bufs=2) as outpool,
            tc.tile_pool(name="ps", bufs=1, space="PSUM") as pspool,
            tc.tile_pool(name="dram", bufs=1, space="DRAM") as dram,
        ):
            # ---- constants ----
            bqT_sb = small.tile([HD, HG], fp32)
            bkT_sb = small.tile([HD, HG], fp32)
            bv_sb = small.tile([128, F], bf16)
            bo_sb = small.tile([128, D], bf16)
            rotC_sb = small.tile([16, S], bf16)
            rotS_sb = small.tile([16, S], bf16)
            ones_sb = small.tile([128, 1], bf16)
            ones_row_sb = small.tile([1, 128], fp32)
            nc.sync.dma_start(ones_row_sb[:], ones_row[:])
            nc.sync.dma_start(bqT_sb[:], bqT[:])
            nc.sync.dma_start(bkT_sb[:], bkT[:])
            nc.sync.dma_start(bv_sb[:], bv_bc[:])
            nc.sync.dma_start(bo_sb[:], bo_bc[:])
            nc.sync.dma_start(rotC_sb[:], rotC[:])
            nc.sync.dma_start(rotS_sb[:], rotS[:])
            nc.sync.dma_start(ones_sb[:], ones[:])

            # resident across phase 2->3
            oT_sb = big.tile([128, HG, S], bf16)  # 32KB/part [hd, h, tok]

            # DRAM bounce tensors
            q_dram = dram.tile([HG, 128, S], bf16, name="q_dram")
            k_dram = dram.tile([HG, 128, S], bf16, name="k_dram")
            v_dram = dram.tile([NKC, 128, F], bf16, name="v_dram")

            # ================= Phase 1: QKV projections =================
            def qk_pass(w_dram, bias_sb, dst_dram, which):
                for hg4 in range(4):
                    ps = {}
                    for hi in range(4):
                        for th in range(2):
                            ps[(hi, th)] = pspool.tile(
                                [128, 512], fp32, tag=f"b{hi * 2 + th}",
                                name=f"ps{which}{hi}{th}",
                            )
                    for dc in range(NDC):
                        wt = wpool.tile([128, 512], bf16, tag=f"w{which}", name="wt")
                        nc.sync.dma_start(
                            wt[:],
                            w_dram[dc * 128 : (dc + 1) * 128,
                                   hg4 * 512 : (hg4 + 1) * 512],
                        )
                        xt = xpool.tile([128, S], bf16, tag="xt", name="xt")
                        nc.scalar.dma_start(xt[:], xT[dc * 128 : (dc + 1) * 128, :])
                        for hi in range(4):
                            for th in range(2):
                                nc.tensor.matmul(
                                    ps[(hi, th)][:],
                                    wt[:, hi * 128 : (hi + 1) * 128],
                                    xt[:, th * 512 : (th + 1) * 512],
                                    start=(dc == 0),
                                    stop=(dc == NDC - 1),
                                )
                    for hi in range(4):
                        h = hg4 * 4 + hi
                        qbf = stpool.tile(
                            [128, S], bf16, tag="qbf", bufs=6, name="qbf"
                        )
                        for th in range(2):
                            # single full-tile evac+bias to bf16 on ACT
                            # (PSUM released by this one op; rotary reads
                            # bf16 from qbf and overwrites [0:32] in place)
                            nc.scalar.activation(
                                qbf[:, th * 512 : (th + 1) * 512],
                                ps[(hi, th)][:],
                                Act.Identity,
                                bias=bias_sb[:, h : h + 1],
                            )
                        # rotary (block order): u=qbf[0:16], w=qbf[16:32].
                        # Engine ops need 32-aligned partition bases, so the
                        # w half bounces via DMA to a base-0 tile and the f
                        # result bounces back to partitions 16:32.
                        rot_w = tmppool.tile([16, S], bf16, tag="rw", name="rot_w")
                        nc.scalar.dma_start(rot_w[:], qbf[16:32, :])
                        t1 = tmppool.tile([16, S], bf16, tag="t1", name="t1")
                        t2 = tmppool.tile([16, S], bf16, tag="t2", name="t2")
                        t3 = tmppool.tile([16, S], bf16, tag="t3", name="t3")
                        t4 = tmppool.tile([16, S], bf16, tag="t4", name="t4")
                        fbuf = tmppool.tile([16, S], bf16, tag="fb", name="fbuf")
                        u = qbf[0:16, :]
                        nc.vector.tensor_mul(t1[:], u, rotC_sb[:])
                        nc.vector.tensor_mul(t3[:], rot_w[:], rotS_sb[:])
                        nc.gpsimd.tensor_mul(t2[:], u, rotS_sb[:])
                        nc.gpsimd.tensor_mul(t4[:], rot_w[:], rotC_sb[:])
                        nc.vector.tensor_sub(qbf[0:16, :], t1[:], t3[:])
                        nc.gpsimd.tensor_add(fbuf[:], t2[:], t4[:])
                        nc.gpsimd.dma_start(qbf[16:32, :], fbuf[:])
                        nc.gpsimd.dma_start(dst_dram[h], qbf[:])

            qk_pass(wq, bqT_sb, q_dram, "q")
            qk_pass(wk, bkT_sb, k_dram, "k")

            # V pass (token-major): psum[tok=128, feat=512]
            for fh in range(2):
                for tq4 in range(2):
                    ps = {}
                    for ti in range(4):
                        for fc in range(2):
                            ps[(ti, fc)] = pspool.tile(
                                [128, 512], fp32, tag=f"b{ti * 2 + fc}",
                                name=f"psv{ti}{fc}",
                            )
                    for dc in range(NDC):
                        wt = wvpool.tile([128, 1024], bf16, tag="wv", name="wvt")
                        nc.sync.dma_start(
                            wt[:],
                            wv[dc * 128 : (dc + 1) * 128,
                               fh * 1024 : (fh + 1) * 1024],
                        )
                        xt = xpool.tile([128, S], bf16, tag="xt", name="xt")
                        nc.scalar.dma_start(xt[:], xT[dc * 128 : (dc + 1) * 128, :])
                        for ti in range(4):
                            tci = tq4 * 4 + ti
                            for fc in range(2):
                                nc.tensor.matmul(
                                    ps[(ti, fc)][:],
                                    xt[:, tci * 128 : (tci + 1) * 128],
                                    wt[:, fc * 512 : (fc + 1) * 512],
                                    start=(dc == 0),
                                    stop=(dc == NDC - 1),
                                )
                    for ti in range(4):
                        tci = tq4 * 4 + ti
                        for fc in range(2):
                            f0 = fh * 1024 + fc * 512
                            vb = evacpool.tile([128, 512], bf16, tag="vb", name="vb")
                            nc.vector.tensor_add(
                                vb[:], ps[(ti, fc)][:], bv_sb[:, f0 : f0 + 512]
                            )
                            nc.gpsimd.dma_start(v_dram[tci, :, f0 : f0 + 512], vb[:])

            # ================= Phase 2: attention per head =================
            for h in range(HG):
                qh_t = stream.tile([128, S], bf16, tag="qh", name="qh_t")
                kh_t = stream.tile([128, S], bf16, tag="kh", name="kh_t")
                vh_t = stream.tile([128, NKC, HD], bf16, tag="vh", name="vh_t")
                nc.sync.dma_start(qh_t[:], q_dram[h])
                nc.sync.dma_start(kh_t[:], k_dram[h])
                nc.sync.dma_start(
                    vh_t[:],
                    v_dram[:, :, h * 128 : (h + 1) * 128].rearrange(
                        "kc p hd -> p kc hd"
                    ),
                )
                E = epool.tile([128, NKC, S], bf16, tag="E", bufs=2, name="E")
                sum_ps = {}
                o_ps = {}
                for qh in range(2):
                    sum_ps[qh] = pspool.tile(
                        [1, 512], fp32, tag=f"b{2 + qh}", name=f"sum{qh}"
                    )
                    o_ps[qh] = pspool.tile(
                        [128, 512], fp32, tag=f"b{4 + qh}", name=f"o{qh}"
                    )
                for kc in range(NKC):
                    bt = stream.tile([128, S], bf16, tag="bt", name="bt")
                    nc.sync.dma_start(bt[:], biasT[kc * 128 : (kc + 1) * 128, :])
                    for qh in range(2):
                        qs = slice(qh * 512, (qh + 1) * 512)
                        stag = (0, 1, 6, 7)[(kc * 2 + qh) % 4]
                        sps = pspool.tile(
                            [128, 512], fp32, tag=f"b{stag}", name="sps"
                        )
                        nc.tensor.matmul(
                            sps[:],
                            kh_t[:, kc * 128 : (kc + 1) * 128],
                            qh_t[:, qs],
                            start=True,
                            stop=True,
                        )
                        nc.vector.tensor_add(E[:, kc, qs], sps[:], bt[:, qs])
                        nc.scalar.activation(
                            E[:, kc, qs], E[:, kc, qs], Act.Exp
                        )
                        # sum and o both consume E chunk-by-chunk, so the
                        # PE never waits for the whole row of E
                        nc.tensor.matmul(
                            sum_ps[qh][:],
                            ones_sb[:],
                            E[:, kc, qs],
                            start=(kc == 0),
                            stop=(kc == NKC - 1),
                        )
                        nc.tensor.matmul(
                            o_ps[qh][:],
                            vh_t[:, kc, :],
                            E[:, kc, qs],
                            start=(kc == 0),
                            stop=(kc == NKC - 1),
                        )
                inv_row = tmppool.tile(
                    [1, S], fp32, tag="inv", bufs=2, name="inv_row"
                )
                inv_bc = tmppool.tile(
                    [128, S], fp32, tag="invbc", bufs=2, name="inv_bc"
                )
                for qh in range(2):
                    nc.vector.reciprocal_approx_fast(
                        inv_row[:, qh * 512 : (qh + 1) * 512], sum_ps[qh][:]
                    )
                for qh in range(2):
                    qs = slice(qh * 512, (qh + 1) * 512)
                    # rank-1 PE broadcast: ones[128] x inv_row -> [128, 512]
                    bc_ps = pspool.tile(
                        [128, 512], fp32, tag=f"b{2 + qh}", name="bc_ps"
                    )
                    nc.tensor.matmul(
                        bc_ps[:],
                        ones_row_sb[:],
                        inv_row[:, qs],
                        start=True,
                        stop=True,
                    )
                    nc.vector.tensor_copy(inv_bc[:, qs], bc_ps[:])
                    nc.vector.tensor_mul(
                        oT_sb[:, h, qs], o_ps[qh][:], inv_bc[:, qs]
                    )

            # ========= Phase 3: out-projection + chunked ReduceScatter =========
            for tq in range(4):
                rs_in = dram.tile([256, D], bf16, tag="rsin", bufs=2, name="rs_in")
                rs_out = dram.tile([128, D], bf16, tag="rsout", bufs=2, name="rs_out")
                for mh in range(2):
                    ps = {}
                    for tc2 in range(2):
                        for mc2 in range(4):
                            ps[(tc2, mc2)] = pspool.tile(
                                [128, 512], fp32, tag=f"b{tc2 * 4 + mc2}",
                                name=f"pso{tc2}{mc2}",
                            )
                    for cc in range(HG):
                        wt = wvpool.tile(
                            [128, 2048], bf16, tag="wo", bufs=4, name="wot"
                        )
                        nc.sync.dma_start(
                            wt[:],
                            wo[cc * 128 : (cc + 1) * 128,
                               mh * 2048 : (mh + 1) * 2048],
                        )
                        for tc2 in range(2):
                            t128 = tq * 2 + tc2
                            for mc2 in range(4):
                                nc.tensor.matmul(
                                    ps[(tc2, mc2)][:],
                                    oT_sb[:, cc, t128 * 128 : (t128 + 1) * 128],
                                    wt[:, mc2 * 512 : (mc2 + 1) * 512],
                                    start=(cc == 0),
                                    stop=(cc == HG - 1),
                                )
                    for tc2 in range(2):
                        po = outpool.tile([128, 2048], bf16, tag="po", name="po")
                        for mc2 in range(4):
                            m0 = mh * 2048 + mc2 * 512
                            nc.vector.tensor_add(
                                po[:, mc2 * 512 : (mc2 + 1) * 512],
                                ps[(tc2, mc2)][:],
                                bo_sb[:, m0 : m0 + 512],
                            )
                        nc.gpsimd.dma_start(
                            rs_in[tc2 * 128 : (tc2 + 1) * 128,
                                  mh * 2048 : (mh + 1) * 2048],
                            po[:],
                        )
                nc.gpsimd.collective_compute(
                    "ReduceScatter",
                    mybir.AluOpType.add,
                    replica_groups=RG,
                    ins=[rs_in[:].opt()],
                    outs=[rs_out[:].opt()],
                )
                for dh in range(4):
                    fin_bf = evacpool.tile(
                        [128, 1024], bf16, tag="finbf", name="fin_bf"
                    )
                    fin_f32 = evacpool.tile(
                        [128, 1024], fp32, tag="finf32", name="fin_f32"
                    )
                    nc.scalar.dma_start(
                        fin_bf[:], rs_out[:, dh * 1024 : (dh + 1) * 1024]
                    )
                    nc.vector.tensor_copy(fin_f32[:], fin_bf[:])
                    nc.scalar.dma_start(
                        out[tq, :, dh * 1024 : (dh + 1) * 1024], fin_f32[:]
                    )

    nc.finalize()
    return nc


def _prep_shards(x, attn_bias, wq_kernel, wq_bias, wk_kernel, wk_bias,
                 wv_kernel, wv_bias, wo_kernel, wo_bias):
    """Host-side shard prep. Returns in_maps (list of 8 dicts)."""
    freqs = 1.0 / 10000.0 ** (np.arange(0, ROTARY, 2) / ROTARY)  # [16]
    pos = np.arange(MAX_POS - S, MAX_POS)  # [S]
    ang = np.outer(freqs, pos)  # [16, S]
    rotC = np.cos(ang).astype(np.float32)
    rotS = np.sin(ang).astype(np.float32)
    ones = np.ones((128, 1), dtype=BF16)
    biasT = np.ascontiguousarray(attn_bias[0, 0].T).astype(BF16)

    in_maps = []
    for c in range(NCORES):
        b, g = c // 2, c % 2
        hs = slice(g * HG, (g + 1) * HG)
        in_maps.append(
            {
                "xT": np.ascontiguousarray(x[b].T).astype(BF16),
                "wq": (wq_kernel[:, hs, :].reshape(D, F) * SCALE).astype(BF16),
                "wk": wk_kernel[:, hs, :].reshape(D, F).astype(BF16),
                "wv": wv_kernel[:, hs, :].reshape(D, F).astype(BF16),
                "wo": wo_kernel[hs].reshape(F, D).astype(BF16),
                "bqT": np.ascontiguousarray((wq_bias[hs] * SCALE).T).astype(
                    np.float32
                ),
                "bkT": np.ascontiguousarray(wk_bias[hs].T).astype(np.float32),
                "bv_bc": np.broadcast_to(
                    wv_bias[hs].reshape(1, F), (128, F)
                ).astype(BF16).copy(),
                "bo_bc": np.broadcast_to(
                    (wo_bias * 0.5).reshape(1, D), (128, D)
                ).astype(BF16).copy(),
                "biasT": biasT,
                "rotC": rotC.astype(BF16),
                "rotS": rotS.astype(BF16),
                "ones": ones,
                "ones_row": np.ones((1, 128), dtype=np.float32),
            }
        )
    return in_maps


def kernel(x, attn_bias, wq_kernel, wq_bias, wk_kernel, wk_bias,
           wv_kernel, wv_bias, wo_kernel, wo_bias, _trace=False):
    from concourse import bass_utils

    if "nc" not in _cache:
        _cache["nc"] = _build()
    nc = _cache["nc"]

    in_maps = _prep_shards(
        np.asarray(x), np.asarray(attn_bias),
        np.asarray(wq_kernel), np.asarray(wq_bias),
        np.asarray(wk_kernel), np.asarray(wk_bias),
        np.asarray(wv_kernel), np.asarray(wv_bias),
        np.asarray(wo_kernel), np.asarray(wo_bias),
    )
    res = bass_utils.run_bass_kernel_spmd(
        nc, in_maps, core_ids=list(range(NCORES)), trace=_trace
    )
    _cache["last_results"] = res

    full = np.empty((B, S, D), dtype=np.float32)
    for b in range(B):
        lo = res.results[2 * b]["out"]  # [4, 128, D]: tokens tq*256 .. +128
        hi = res.results[2 * b + 1]["out"]  # tokens tq*256+128 .. +256
        for tq in range(4):
            full[b, tq * 256 : tq * 256 + 128] = lo[tq]
            full[b, tq * 256 + 128 : (tq + 1) * 256] = hi[tq]
    return full


# revision 26
# speedup vs baseline: 1.2421x; 1.2111x over previous
"""Distributed Bass kernel for attention (B=4,S=1024,D=4096,H=32,HD=128).

Sharding: 8 cores = 4 batch x 2 head-groups of 16 heads (core c: batch c//2,
heads (c%2)*16..+16). Per-core pipeline (all matmuls bf16 with fp32 PSUM):

  1. QKV projections. q/k computed feature-major (q^T[hd,tok] per head) so
     attention needs no transposes; v computed token-major. x and weights are
     streamed; q/k get partial rotary applied in-SBUF then bounce via DRAM.
     Rotary uses the block-order trick: scores are invariant to a channel
     permutation applied identically to q and k, so the rotated (real,imag)
     halves stay block-contiguous instead of interleaved.
  2. Scores computed directly transposed E[k,q] = exp(kT.T @ qT + biasT).
     No max subtraction (scores bounded for this input distribution); the
     softmax denominator is a ones-vector matmul reducing over partitions.
     1/s is folded into the o^T PSUM evacuation (o^T = v.T @ E unnormalized).
  3. Output projection partial sums with wo_bias/2 folded in, chunked
     pairwise ReduceScatter (bf16) over token quarters, fp32 cast, DMA out.
"""

import sys

sys.path.insert(0, "/opt/trn_rl_repo")

import numpy as np
import ml_dtypes

BF16 = ml_dtypes.bfloat16

B, S, D, H, HD = 4, 1024, 4096, 32, 128
ROTARY = 32
MAX_POS = 10000
HG = H // 2  # heads per core = 16
F = HG * HD  # per-core qkv feature dim = 2048
NCORES = 8
SCALE = 1.0 / np.sqrt(HD)
NDC = D // 128  # 32 contraction chunks
NKC = S // 128  # 8 key chunks

_cache = {}


def _build():
    import concourse.mybir as mybir
    import concourse.tile as tile
    from concourse import bacc

    fp32 = mybir.dt.float32
    bf16 = mybir.dt.bfloat16
    Act = mybir.ActivationFunctionType

    nc = bacc.Bacc("TRN2", target_bir_lowering=False, num_devices=NCORES)

    # ---- DRAM parameters (per-core shards) ----
    xT = nc.dram_tensor("xT", [D, S], bf16, kind="ExternalInput")
    wq = nc.dram_tensor("wq", [D, F], bf16, kind="ExternalInput")
    wk = nc.dram_tensor("wk", [D, F], bf16, kind="ExternalInput")
    wv = nc.dram_tensor("wv", [D, F], bf16, kind="ExternalInput")
    wo = nc.dram_tensor("wo", [F, D], bf16, kind="ExternalInput")
    bqT = nc.dram_tensor("bqT", [HD, HG], fp32, kind="ExternalInput")
    bkT = nc.dram_tensor("bkT", [HD, HG], fp32, kind="ExternalInput")
    bv_bc = nc.dram_tensor("bv_bc", [128, F], bf16, kind="ExternalInput")
    bo_bc = nc.dram_tensor("bo_bc", [128, D], bf16, kind="ExternalInput")
    biasT = nc.dram_tensor("biasT", [S, S], bf16, kind="ExternalInput")
    rotC = nc.dram_tensor("rotC", [16, S], bf16, kind="ExternalInput")
    rotS = nc.dram_tensor("rotS", [16, S], bf16, kind="ExternalInput")
    ones = nc.dram_tensor("ones", [128, 1], bf16, kind="ExternalInput")
    ones_row = nc.dram_tensor("ones_row", [1, 128], fp32, kind="ExternalInput")
    out = nc.dram_tensor("out", [4, 128, D], fp32, kind="ExternalOutput")

    RG = [[0, 1], [2, 3], [4, 5], [6, 7]]

    with tile.TileContext(nc) as tc:
        with (
            tc.tile_pool(name="wpool", bufs=3) as wpool,
            tc.tile_pool(name="wvpool", bufs=2) as wvpool,
            tc.tile_pool(name="xpool", bufs=2) as xpool,
            tc.tile_pool(name="stream", bufs=2) as stream,
            tc.tile_pool(name="stage", bufs=2) as stpool,
            tc.tile_pool(name="tmp", bufs=1) as tmppool,
            tc.tile_pool(name="small", bufs=1) as small,
            tc.tile_pool(name="epool", bufs=1) as epool,
            tc.tile_pool(name="big", bufs=1) as big,
            tc.tile_pool(name="evac", bufs=2) as evacpool,
            tc.tile_pool(name="outp", bufs=2) as outpool,
            tc.tile_pool(name="ps", bufs=1, space="PSUM") as pspool,
            tc.tile_pool(name="dram", bufs=1, space="DRAM") as dram,
        ):
            # ---- constants ----
            bqT_sb = small.tile([HD, HG], fp32)
            bkT_sb = small.tile([HD, HG], fp32)
            bv_sb = small.tile([128, F], bf16)
            bo_sb = small.tile([128, D], bf16)
            rotC_sb = small.tile([16, S], bf16)
            rotS_sb = small.tile([16, S], bf16)
            ones_sb = small.tile([128, 1], bf16)
            ones_row_sb = small.tile([1, 128], fp32)
            nc.sync.dma_start(ones_row_sb[:], ones_row[:])
            nc.sync.dma_start(bqT_sb[:], bqT[:])
            nc.sync.dma_start(bkT_sb[:], bkT[:])
            nc.sync.dma_start(bv_sb[:], bv_bc[:])
            nc.sync.dma_start(bo_sb[:], bo_bc[:])
            nc.sync.dma_start(rotC_sb[:], rotC[:])
            nc.sync.dma_start(rotS_sb[:], rotS[:])
            nc.sync.dma_start(ones_sb[:], ones[:])

            # resident input activations [d, dc, tok] (64KB/part)
            xT_sb = big.tile([128, NDC, S], bf16, name="xT_sb")
            for i in range(4):
                nc.sync.dma_start(
                    xT_sb[:, i * 8 : (i + 1) * 8, :],
                    xT[i * 1024 : (i + 1) * 1024, :].rearrange(
                        "(a p) t -> p a t", p=128
                    ),
                )

            # DRAM bounce tensors
            q_dram = dram.tile([HG, 128, S], bf16, name="q_dram")
            k_dram = dram.tile([HG, 128, S], bf16, name="k_dram")
            v_dram = dram.tile([NKC, 128, F], bf16, name="v_dram")
            oT_dram = dram.tile([HG, 128, S], bf16, name="oT_dram")

            # ================= Phase 1: QKV projections =================
            def qk_pass(w_dram, bias_sb, dst_dram, which):
                for hg4 in range(4):
                    ps = {}
                    for hi in range(4):
                        for th in range(2):
                            ps[(hi, th)] = pspool.tile(
                                [128, 512], fp32, tag=f"b{hi * 2 + th}",
                                name=f"ps{which}{hi}{th}",
                            )
                    for dc in range(NDC):
                        wt = wpool.tile([128, 512], bf16, tag=f"w{which}", name="wt")
                        nc.sync.dma_start(
                            wt[:],
                            w_dram[dc * 128 : (dc + 1) * 128,
                                   hg4 * 512 : (hg4 + 1) * 512],
                        )
                        for hi in range(4):
                            for th in range(2):
                                nc.tensor.matmul(
                                    ps[(hi, th)][:],
                                    wt[:, hi * 128 : (hi + 1) * 128],
                                    xT_sb[:, dc, th * 512 : (th + 1) * 512],
                                    start=(dc == 0),
                                    stop=(dc == NDC - 1),
                                )
                    for hi in range(4):
                        h = hg4 * 4 + hi
                        qbf = stpool.tile(
                            [128, S], bf16, tag="qbf", bufs=5, name="qbf"
                        )
                        for th in range(2):
                            # single full-tile evac+bias to bf16 on ACT
                            # (PSUM released by this one op; rotary reads
                            # bf16 from qbf and overwrites [0:32] in place)
                            nc.scalar.activation(
                                qbf[:, th * 512 : (th + 1) * 512],
                                ps[(hi, th)][:],
                                Act.Identity,
                                bias=bias_sb[:, h : h + 1],
                            )
                        # rotary (block order): u=qbf[0:16], w=qbf[16:32].
                        # Engine ops need 32-aligned partition bases, so the
                        # w half bounces via DMA to a base-0 tile and the f
                        # result bounces back to partitions 16:32.
                        rot_w = tmppool.tile([16, S], bf16, tag="rw", name="rot_w")
                        nc.scalar.dma_start(rot_w[:], qbf[16:32, :])
                        t1 = tmppool.tile([16, S], bf16, tag="t1", name="t1")
                        t2 = tmppool.tile([16, S], bf16, tag="t2", name="t2")
                        t3 = tmppool.tile([16, S], bf16, tag="t3", name="t3")
                        t4 = tmppool.tile([16, S], bf16, tag="t4", name="t4")
                        fbuf = tmppool.tile([16, S], bf16, tag="fb", name="fbuf")
                        u = qbf[0:16, :]
                        nc.vector.tensor_mul(t1[:], u, rotC_sb[:])
                        nc.vector.tensor_mul(t3[:], rot_w[:], rotS_sb[:])
                        nc.gpsimd.tensor_mul(t2[:], u, rotS_sb[:])
                        nc.gpsimd.tensor_mul(t4[:], rot_w[:], rotC_sb[:])
                        nc.vector.tensor_sub(qbf[0:16, :], t1[:], t3[:])
                        nc.gpsimd.tensor_add(fbuf[:], t2[:], t4[:])
                        nc.gpsimd.dma_start(qbf[16:32, :], fbuf[:])
                        nc.gpsimd.dma_start(dst_dram[h], qbf[:])

            qk_pass(wq, bqT_sb, q_dram, "q")
            qk_pass(wk, bkT_sb, k_dram, "k")

            # V pass (token-major): psum[tok=128, feat=512]
            for fh in range(2):
                for tq4 in range(2):
                    ps = {}
                    for ti in range(4):
                        for fc in range(2):
                            ps[(ti, fc)] = pspool.tile(
                                [128, 512], fp32, tag=f"b{ti * 2 + fc}",
                                name=f"psv{ti}{fc}",
                            )
                    for dc in range(NDC):
                        wt = wvpool.tile([128, 1024], bf16, tag="wv", name="wvt")
                        nc.sync.dma_start(
                            wt[:],
                            wv[dc * 128 : (dc + 1) * 128,
                               fh * 1024 : (fh + 1) * 1024],
                        )
                        for ti in range(4):
                            tci = tq4 * 4 + ti
                            for fc in range(2):
                                nc.tensor.matmul(
                                    ps[(ti, fc)][:],
                                    xT_sb[:, dc, tci * 128 : (tci + 1) * 128],
                                    wt[:, fc * 512 : (fc + 1) * 512],
                                    start=(dc == 0),
                                    stop=(dc == NDC - 1),
                                )
                    for ti in range(4):
                        tci = tq4 * 4 + ti
                        for fc in range(2):
                            f0 = fh * 1024 + fc * 512
                            vb = evacpool.tile([128, 512], bf16, tag="vb", name="vb")
                            nc.vector.tensor_add(
                                vb[:], ps[(ti, fc)][:], bv_sb[:, f0 : f0 + 512]
                            )
                            nc.gpsimd.dma_start(v_dram[tci, :, f0 : f0 + 512], vb[:])

            # ================= Phase 2: attention per head =================
            for h in range(HG):
                qh_t = stream.tile([128, S], bf16, tag="qh", name="qh_t")
                kh_t = stream.tile([128, S], bf16, tag="kh", name="kh_t")
                vh_t = stream.tile([128, NKC, HD], bf16, tag="vh", name="vh_t")
                nc.sync.dma_start(qh_t[:], q_dram[h])
                nc.sync.dma_start(kh_t[:], k_dram[h])
                nc.sync.dma_start(
                    vh_t[:],
                    v_dram[:, :, h * 128 : (h + 1) * 128].rearrange(
                        "kc p hd -> p kc hd"
                    ),
                )
                E = epool.tile([128, NKC, S], bf16, tag="E", bufs=1, name="E")
                oTh = stream.tile([128, S], bf16, tag="oTh", bufs=2, name="oTh")
                sum_ps = {}
                o_ps = {}
                for qh in range(2):
                    sum_ps[qh] = pspool.tile(
                        [1, 512], fp32, tag=f"b{2 + qh}", name=f"sum{qh}"
                    )
                    o_ps[qh] = pspool.tile(
                        [128, 512], fp32, tag=f"b{4 + qh}", name=f"o{qh}"
                    )
                for kc in range(NKC):
                    bt = stream.tile([128, S], bf16, tag="bt", name="bt")
                    nc.sync.dma_start(bt[:], biasT[kc * 128 : (kc + 1) * 128, :])
                    for qh in range(2):
                        qs = slice(qh * 512, (qh + 1) * 512)
                        stag = (0, 1, 6, 7)[(kc * 2 + qh) % 4]
                        sps = pspool.tile(
                            [128, 512], fp32, tag=f"b{stag}", name="sps"
                        )
                        nc.tensor.matmul(
                            sps[:],
                            kh_t[:, kc * 128 : (kc + 1) * 128],
                            qh_t[:, qs],
                            start=True,
                            stop=True,
                        )
                        nc.vector.tensor_add(E[:, kc, qs], sps[:], bt[:, qs])
                        nc.scalar.activation(
                            E[:, kc, qs], E[:, kc, qs], Act.Exp
                        )
                        # sum and o both consume E chunk-by-chunk, so the
                        # PE never waits for the whole row of E
                        nc.tensor.matmul(
                            sum_ps[qh][:],
                            ones_sb[:],
                            E[:, kc, qs],
                            start=(kc == 0),
                            stop=(kc == NKC - 1),
                        )
                        nc.tensor.matmul(
                            o_ps[qh][:],
                            vh_t[:, kc, :],
                            E[:, kc, qs],
                            start=(kc == 0),
                            stop=(kc == NKC - 1),
                        )
                inv_row = tmppool.tile(
                    [1, S], fp32, tag="inv", bufs=2, name="inv_row"
                )
                inv_bc = tmppool.tile(
                    [128, S], fp32, tag="invbc", bufs=1, name="inv_bc"
                )
                for qh in range(2):
                    nc.vector.reciprocal_approx_fast(
                        inv_row[:, qh * 512 : (qh + 1) * 512], sum_ps[qh][:]
                    )
                for qh in range(2):
                    qs = slice(qh * 512, (qh + 1) * 512)
                    # rank-1 PE broadcast: ones[128] x inv_row -> [128, 512]
                    bc_ps = pspool.tile(
                        [128, 512], fp32, tag=f"b{2 + qh}", name="bc_ps"
                    )
                    nc.tensor.matmul(
                        bc_ps[:],
                        ones_row_sb[:],
                        inv_row[:, qs],
                        start=True,
                        stop=True,
                    )
                    nc.vector.tensor_copy(inv_bc[:, qs], bc_ps[:])
                    nc.vector.tensor_mul(oTh[:, qs], o_ps[qh][:], inv_bc[:, qs])
                nc.gpsimd.dma_start(oT_dram[h], oTh[:])

            # ========= Phase 3: out-projection + chunked ReduceScatter =========
            for tq in range(4):
                rs_in = dram.tile([256, D], bf16, tag="rsin", bufs=2, name="rs_in")
                rs_out = dram.tile([128, D], bf16, tag="rsout", bufs=2, name="rs_out")
                otl = {}
                for tc2 in range(2):
                    t128 = tq * 2 + tc2
                    otl[tc2] = stream.tile(
                        [128, HG, 128], bf16, tag="otl", bufs=2,
                        name=f"otl{tc2}",
                    )
                    nc.scalar.dma_start(
                        otl[tc2][:],
                        oT_dram[:, :, t128 * 128 : (t128 + 1) * 128].rearrange(
                            "h p t -> p h t"
                        ),
                    )
                for mh in range(2):
                    ps = {}
                    for tc2 in range(2):
                        for mc2 in range(4):
                            ps[(tc2, mc2)] = pspool.tile(
                                [128, 512], fp32, tag=f"b{tc2 * 4 + mc2}",
                                name=f"pso{tc2}{mc2}",
                            )
                    for cc in range(HG):
                        wt = wvpool.tile(
                            [128, 2048], bf16, tag="wo", bufs=4, name="wot"
                        )
                        nc.sync.dma_start(
                            wt[:],
                            wo[cc * 128 : (cc + 1) * 128,
                               mh * 2048 : (mh + 1) * 2048],
                        )
                        for tc2 in range(2):
                            for mc2 in range(4):
                                nc.tensor.matmul(
                                    ps[(tc2, mc2)][:],
                                    otl[tc2][:, cc, :],
                                    wt[:, mc2 * 512 : (mc2 + 1) * 512],
                                    start=(cc == 0),
                                    stop=(cc == HG - 1),
                                )
                    for tc2 in range(2):
                        po = outpool.tile([128, 2048], bf16, tag="po", name="po")
                        for mc2 in range(4):
                            m0 = mh * 2048 + mc2 * 512
                            nc.vector.tensor_add(
                                po[:, mc2 * 512 : (mc2 + 1) * 512],
                                ps[(tc2, mc2)][:],
                                bo_sb[:, m0 : m0 + 512],
                            )
                        nc.gpsimd.dma_start(
                            rs_in[tc2 * 128 : (tc2 + 1) * 128,
                                  mh * 2048 : (mh + 1) * 2048],
                            po[:],
                        )
                nc.gpsimd.collective_compute(
                    "ReduceScatter",
                    mybir.AluOpType.add,
                    replica_groups=RG,
                    ins=[rs_in[:].opt()],
                    outs=[rs_out[:].opt()],
                )
                for dh in range(4):
                    fin_bf = evacpool.tile(
                        [128, 1024], bf16, tag="finbf", name="fin_bf"
                    )
                    fin_f32 = evacpool.tile(
                        [128, 1024], fp32, tag="finf32", name="fin_f32"
                    )
                    nc.scalar.dma_start(
                        fin_bf[:], rs_out[:, dh * 1024 : (dh + 1) * 1024]
                    )
                    nc.vector.tensor_copy(fin_f32[:], fin_bf[:])
                    nc.scalar.dma_start(
                        out[tq, :, dh * 1024 : (dh + 1) * 1024], fin_f32[:]
                    )

    nc.finalize()
    return nc


def _prep_shards(x, attn_bias, wq_kernel, wq_bias, wk_kernel, wk_bias,
                 wv_kernel, wv_bias, wo_kernel, wo_bias):
    """Host-side shard prep. Returns in_maps (list of 8 dicts)."""
    freqs = 1.0 / 10000.0 ** (np.arange(0, ROTARY, 2) / ROTARY)  # [16]
    pos = np.arange(MAX_POS - S, MAX_POS)  # [S]
    ang = np.outer(freqs, pos)  # [16, S]
    rotC = np.cos(ang).astype(np.float32)
    rotS = np.sin(ang).astype(np.float32)
    ones = np.ones((128, 1), dtype=BF16)
    biasT = np.ascontiguousarray(attn_bias[0, 0].T).astype(BF16)

    in_maps = []
    for c in range(NCORES):
        b, g = c // 2, c % 2
        hs = slice(g * HG, (g + 1) * HG)
        in_maps.append(
            {
                "xT": np.ascontiguousarray(x[b].T).astype(BF16),
                "wq": (wq_kernel[:, hs, :].reshape(D, F) * SCALE).astype(BF16),
                "wk": wk_kernel[:, hs, :].reshape(D, F).astype(BF16),
                "wv": wv_kernel[:, hs, :].reshape(D, F).astype(BF16),
                "wo": wo_kernel[hs].reshape(F, D).astype(BF16),
                "bqT": np.ascontiguousarray((wq_bias[hs] * SCALE).T).astype(
                    np.float32
                ),
                "bkT": np.ascontiguousarray(wk_bias[hs].T).astype(np.float32),
                "bv_bc": np.broadcast_to(
                    wv_bias[hs].reshape(1, F), (128, F)
                ).astype(BF16).copy(),
                "bo_bc": np.broadcast_to(
                    (wo_bias * 0.5).reshape(1, D), (128, D)
                ).astype(BF16).copy(),
                "biasT": biasT,
                "rotC": rotC.astype(BF16),
                "rotS": rotS.astype(BF16),
                "ones": ones,
                "ones_row": np.ones((1, 128), dtype=np.float32),
            }
        )
    return in_maps


def kernel(x, attn_bias, wq_kernel, wq_bias, wk_kernel, wk_bias,
           wv_kernel, wv_bias, wo_kernel, wo_bias, _trace=False):
    from concourse import bass_utils

    if "nc" not in _cache:
        _cache["nc"] = _build()
    nc = _cache["nc"]

    in_maps = _prep_shards(
        np.asarray(x), np.asarray(attn_bias),
        np.asarray(wq_kernel), np.asarray(wq_bias),
        np.asarray(wk_kernel), np.asarray(wk_bias),
        np.asarray(wv_kernel), np.asarray(wv_bias),
        np.asarray(wo_kernel), np.asarray(wo_bias),
    )
    res = bass_utils.run_bass_kernel_spmd(
        nc, in_maps, core_ids=list(range(NCORES)), trace=_trace
    )
    _cache["last_results"] = res

    full = np.empty((B, S, D), dtype=np.float32)
    for b in range(B):
        lo = res.results[2 * b]["out"]  # [4, 128, D]: tokens tq*256 .. +128
        hi = res.results[2 * b + 1]["out"]  # tokens tq*256+128 .. +256
        for tq in range(4):
            full[b, tq * 256 : tq * 256 + 128] = lo[tq]
            full[b, tq * 256 + 128 : (tq + 1) * 256] = hi[tq]
    return full


# revision 27
# speedup vs baseline: 1.3094x; 1.0542x over previous
"""Distributed Bass kernel for attention (B=4,S=1024,D=4096,H=32,HD=128).

Sharding: 8 cores = 4 batch x 2 head-groups of 16 heads (core c: batch c//2,
heads (c%2)*16..+16). Per-core pipeline (all matmuls bf16 with fp32 PSUM):

  1. QKV projections. q/k computed feature-major (q^T[hd,tok] per head) so
     attention needs no transposes; v computed token-major. x and weights are
     streamed; q/k get partial rotary applied in-SBUF then bounce via DRAM.
     Rotary uses the block-order trick: scores are invariant to a channel
     permutation applied identically to q and k, so the rotated (real,imag)
     halves stay block-contiguous instead of interleaved.
  2. Scores computed directly transposed E[k,q] = exp(kT.T @ qT + biasT).
     No max subtraction (scores bounded for this input distribution); the
     softmax denominator is a ones-vector matmul reducing over partitions.
     1/s is folded into the o^T PSUM evacuation (o^T = v.T @ E unnormalized).
  3. Output projection partial sums with wo_bias/2 folded in, chunked
     pairwise ReduceScatter (bf16) over token quarters, fp32 cast, DMA out.
"""

import sys

sys.path.insert(0, "/opt/trn_rl_repo")

import numpy as np
import ml_dtypes

BF16 = ml_dtypes.bfloat16

B, S, D, H, HD = 4, 1024, 4096, 32, 128
ROTARY = 32
MAX_POS = 10000
HG = H // 2  # heads per core = 16
F = HG * HD  # per-core qkv feature dim = 2048
NCORES = 8
SCALE = 1.0 / np.sqrt(HD)
NDC = D // 128  # 32 contraction chunks
NKC = S // 128  # 8 key chunks

_cache = {}


def _build():
    import concourse.mybir as mybir
    import concourse.tile as tile
    from concourse import bacc

    fp32 = mybir.dt.float32
    bf16 = mybir.dt.bfloat16
    Act = mybir.ActivationFunctionType

    nc = bacc.Bacc("TRN2", target_bir_lowering=False, num_devices=NCORES)

    # ---- DRAM parameters (per-core shards) ----
    xT = nc.dram_tensor("xT", [D, S], bf16, kind="ExternalInput")
    wq = nc.dram_tensor("wq", [D, F], bf16, kind="ExternalInput")
    wk = nc.dram_tensor("wk", [D, F], bf16, kind="ExternalInput")
    wv = nc.dram_tensor("wv", [D, F], bf16, kind="ExternalInput")
    wo = nc.dram_tensor("wo", [F, D], bf16, kind="ExternalInput")
    bqT = nc.dram_tensor("bqT", [HD, HG], fp32, kind="ExternalInput")
    bkT = nc.dram_tensor("bkT", [HD, HG], fp32, kind="ExternalInput")
    bv_bc = nc.dram_tensor("bv_bc", [128, F], bf16, kind="ExternalInput")
    bo_bc = nc.dram_tensor("bo_bc", [128, D], bf16, kind="ExternalInput")
    biasT = nc.dram_tensor("biasT", [S, S], bf16, kind="ExternalInput")
    rotC = nc.dram_tensor("rotC", [16, S], bf16, kind="ExternalInput")
    rotS = nc.dram_tensor("rotS", [16, S], bf16, kind="ExternalInput")
    ones = nc.dram_tensor("ones", [128, 1], bf16, kind="ExternalInput")
    ones_row = nc.dram_tensor("ones_row", [1, 128], fp32, kind="ExternalInput")
    out = nc.dram_tensor("out", [4, 128, D], fp32, kind="ExternalOutput")

    RG = [[0, 1], [2, 3], [4, 5], [6, 7]]

    with tile.TileContext(nc) as tc:
        with (
            tc.tile_pool(name="wpool", bufs=3) as wpool,
            tc.tile_pool(name="wvpool", bufs=2) as wvpool,
            tc.tile_pool(name="xpool", bufs=2) as xpool,
            tc.tile_pool(name="stream", bufs=2) as stream,
            tc.tile_pool(name="stage", bufs=2) as stpool,
            tc.tile_pool(name="tmp", bufs=1) as tmppool,
            tc.tile_pool(name="small", bufs=1) as small,
            tc.tile_pool(name="epool", bufs=1) as epool,
            tc.tile_pool(name="big", bufs=1) as big,
            tc.tile_pool(name="evac", bufs=2) as evacpool,
            tc.tile_pool(name="outp", bufs=2) as outpool,
            tc.tile_pool(name="ps", bufs=1, space="PSUM") as pspool,
            tc.tile_pool(name="dram", bufs=1, space="DRAM") as dram,
        ):
            # ---- constants ----
            bqT_sb = small.tile([HD, HG], fp32)
            bkT_sb = small.tile([HD, HG], fp32)
            bv_sb = small.tile([128, F], bf16)
            bo_sb = small.tile([128, D], bf16)
            rotC_sb = small.tile([16, S], bf16)
            rotS_sb = small.tile([16, S], bf16)
            ones_sb = small.tile([128, 1], bf16)
            ones_row_sb = small.tile([1, 128], fp32)
            nc.sync.dma_start(ones_row_sb[:], ones_row[:])
            nc.sync.dma_start(bqT_sb[:], bqT[:])
            nc.sync.dma_start(bkT_sb[:], bkT[:])
            nc.sync.dma_start(bv_sb[:], bv_bc[:])
            nc.sync.dma_start(bo_sb[:], bo_bc[:])
            nc.sync.dma_start(rotC_sb[:], rotC[:])
            nc.sync.dma_start(rotS_sb[:], rotS[:])
            nc.sync.dma_start(ones_sb[:], ones[:])

            # resident input activations [d, dc, tok] (64KB/part);
            # oT_sb later reuses this slot (x is dead after the V pass)
            xT_sb = big.tile([128, NDC, S], bf16, tag="bigbuf", name="xT_sb")
            for i in range(4):
                nc.sync.dma_start(
                    xT_sb[:, i * 8 : (i + 1) * 8, :],
                    xT[i * 1024 : (i + 1) * 1024, :].rearrange(
                        "(a p) t -> p a t", p=128
                    ),
                )

            # DRAM bounce tensors
            q_dram = dram.tile([HG, 128, S], bf16, name="q_dram")
            k_dram = dram.tile([HG, 128, S], bf16, name="k_dram")
            v_dram = dram.tile([NKC, 128, F], bf16, name="v_dram")

            # ================= Phase 1: QKV projections =================
            def qk_pass(w_dram, bias_sb, dst_dram, which):
                for hg4 in range(4):
                    ps = {}
                    for hi in range(4):
                        for th in range(2):
                            ps[(hi, th)] = pspool.tile(
                                [128, 512], fp32, tag=f"b{hi * 2 + th}",
                                name=f"ps{which}{hi}{th}",
                            )
                    for dc in range(NDC):
                        wt = wpool.tile([128, 512], bf16, tag=f"w{which}", name="wt")
                        nc.sync.dma_start(
                            wt[:],
                            w_dram[dc * 128 : (dc + 1) * 128,
                                   hg4 * 512 : (hg4 + 1) * 512],
                        )
                        for hi in range(4):
                            for th in range(2):
                                nc.tensor.matmul(
                                    ps[(hi, th)][:],
                                    wt[:, hi * 128 : (hi + 1) * 128],
                                    xT_sb[:, dc, th * 512 : (th + 1) * 512],
                                    start=(dc == 0),
                                    stop=(dc == NDC - 1),
                                )
                    for hi in range(4):
                        h = hg4 * 4 + hi
                        qbf = stpool.tile(
                            [128, S], bf16, tag="qbf", bufs=5, name="qbf"
                        )
                        for th in range(2):
                            # single full-tile evac+bias to bf16 on ACT
                            # (PSUM released by this one op; rotary reads
                            # bf16 from qbf and overwrites [0:32] in place)
                            nc.scalar.activation(
                                qbf[:, th * 512 : (th + 1) * 512],
                                ps[(hi, th)][:],
                                Act.Identity,
                                bias=bias_sb[:, h : h + 1],
                            )
                        # rotary (block order): u=qbf[0:16], w=qbf[16:32].
                        # Engine ops need 32-aligned partition bases, so the
                        # w half bounces via DMA to a base-0 tile and the f
                        # result bounces back to partitions 16:32.
                        rot_w = tmppool.tile([16, S], bf16, tag="rw", name="rot_w")
                        nc.scalar.dma_start(rot_w[:], qbf[16:32, :])
                        t1 = tmppool.tile([16, S], bf16, tag="t1", name="t1")
                        t2 = tmppool.tile([16, S], bf16, tag="t2", name="t2")
                        t3 = tmppool.tile([16, S], bf16, tag="t3", name="t3")
                        t4 = tmppool.tile([16, S], bf16, tag="t4", name="t4")
                        fbuf = tmppool.tile([16, S], bf16, tag="fb", name="fbuf")
                        u = qbf[0:16, :]
                        nc.vector.tensor_mul(t1[:], u, rotC_sb[:])
                        nc.vector.tensor_mul(t3[:], rot_w[:], rotS_sb[:])
                        nc.gpsimd.tensor_mul(t2[:], u, rotS_sb[:])
                        nc.gpsimd.tensor_mul(t4[:], rot_w[:], rotC_sb[:])
                        nc.vector.tensor_sub(qbf[0:16, :], t1[:], t3[:])
                        nc.gpsimd.tensor_add(fbuf[:], t2[:], t4[:])
                        nc.gpsimd.dma_start(qbf[16:32, :], fbuf[:])
                        nc.gpsimd.dma_start(dst_dram[h], qbf[:])

            qk_pass(wq, bqT_sb, q_dram, "q")
            qk_pass(wk, bkT_sb, k_dram, "k")

            # V pass (token-major): psum[tok=128, feat=512]
            for fh in range(2):
                for tq4 in range(2):
                    ps = {}
                    for ti in range(4):
                        for fc in range(2):
                            ps[(ti, fc)] = pspool.tile(
                                [128, 512], fp32, tag=f"b{ti * 2 + fc}",
                                name=f"psv{ti}{fc}",
                            )
                    for dc in range(NDC):
                        wt = wvpool.tile([128, 1024], bf16, tag="wv", bufs=3, name="wvt")
                        nc.sync.dma_start(
                            wt[:],
                            wv[dc * 128 : (dc + 1) * 128,
                               fh * 1024 : (fh + 1) * 1024],
                        )
                        for ti in range(4):
                            tci = tq4 * 4 + ti
                            for fc in range(2):
                                nc.tensor.matmul(
                                    ps[(ti, fc)][:],
                                    xT_sb[:, dc, tci * 128 : (tci + 1) * 128],
                                    wt[:, fc * 512 : (fc + 1) * 512],
                                    start=(dc == 0),
                                    stop=(dc == NDC - 1),
                                )
                    for ti in range(4):
                        tci = tq4 * 4 + ti
                        for fc in range(2):
                            f0 = fh * 1024 + fc * 512
                            vb = evacpool.tile([128, 512], bf16, tag="vb", name="vb")
                            nc.vector.tensor_add(
                                vb[:], ps[(ti, fc)][:], bv_sb[:, f0 : f0 + 512]
                            )
                            nc.scalar.dma_start(v_dram[tci, :, f0 : f0 + 512], vb[:])

            # ================= Phase 2: attention per head =================
            oT_sb = big.tile(
                [128, HG, S], bf16, tag="bigbuf", name="oT_sb"
            )  # [hd, h, tok]
            for h in range(HG):
                qh_t = stream.tile([128, S], bf16, tag="qh", name="qh_t")
                kh_t = stream.tile([128, S], bf16, tag="kh", name="kh_t")
                vh_t = stream.tile([128, NKC, HD], bf16, tag="vh", name="vh_t")
                nc.sync.dma_start(qh_t[:], q_dram[h])
                nc.sync.dma_start(kh_t[:], k_dram[h])
                nc.sync.dma_start(
                    vh_t[:],
                    v_dram[:, :, h * 128 : (h + 1) * 128].rearrange(
                        "kc p hd -> p kc hd"
                    ),
                )
                E = epool.tile([128, NKC, S], bf16, tag="E", bufs=2, name="E")
                sum_ps = {}
                o_ps = {}
                for qh in range(2):
                    sum_ps[qh] = pspool.tile(
                        [1, 512], fp32, tag=f"b{2 + qh}", name=f"sum{qh}"
                    )
                    o_ps[qh] = pspool.tile(
                        [128, 512], fp32, tag=f"b{4 + qh}", name=f"o{qh}"
                    )
                for kc in range(NKC):
                    bt = stream.tile([128, S], bf16, tag="bt", name="bt")
                    nc.sync.dma_start(bt[:], biasT[kc * 128 : (kc + 1) * 128, :])
                    for qh in range(2):
                        qs = slice(qh * 512, (qh + 1) * 512)
                        stag = (0, 1, 6, 7)[(kc * 2 + qh) % 4]
                        sps = pspool.tile(
                            [128, 512], fp32, tag=f"b{stag}", name="sps"
                        )
                        nc.tensor.matmul(
                            sps[:],
                            kh_t[:, kc * 128 : (kc + 1) * 128],
                            qh_t[:, qs],
                            start=True,
                            stop=True,
                        )
                        nc.vector.tensor_add(E[:, kc, qs], sps[:], bt[:, qs])
                        nc.scalar.activation(
                            E[:, kc, qs], E[:, kc, qs], Act.Exp
                        )
                        # sum and o both consume E chunk-by-chunk, so the
                        # PE never waits for the whole row of E
                        nc.tensor.matmul(
                            sum_ps[qh][:],
                            ones_sb[:],
                            E[:, kc, qs],
                            start=(kc == 0),
                            stop=(kc == NKC - 1),
                        )
                        nc.tensor.matmul(
                            o_ps[qh][:],
                            vh_t[:, kc, :],
                            E[:, kc, qs],
                            start=(kc == 0),
                            stop=(kc == NKC - 1),
                        )
                inv_row = tmppool.tile(
                    [1, S], fp32, tag="inv", bufs=2, name="inv_row"
                )
                inv_bc = tmppool.tile(
                    [128, S], fp32, tag="invbc", bufs=1, name="inv_bc"
                )
                for qh in range(2):
                    nc.vector.reciprocal_approx_fast(
                        inv_row[:, qh * 512 : (qh + 1) * 512], sum_ps[qh][:]
                    )
                for qh in range(2):
                    qs = slice(qh * 512, (qh + 1) * 512)
                    # rank-1 PE broadcast: ones[128] x inv_row -> [128, 512]
                    bc_ps = pspool.tile(
                        [128, 512], fp32, tag=f"b{2 + qh}", name="bc_ps"
                    )
                    nc.tensor.matmul(
                        bc_ps[:],
                        ones_row_sb[:],
                        inv_row[:, qs],
                        start=True,
                        stop=True,
                    )
                    nc.vector.tensor_copy(inv_bc[:, qs], bc_ps[:])
                    nc.vector.tensor_mul(
                        oT_sb[:, h, qs], o_ps[qh][:], inv_bc[:, qs]
                    )

            # ========= Phase 3: out-projection + chunked ReduceScatter =========
            for tq in range(4):
                rs_in = dram.tile([256, D], bf16, tag="rsin", bufs=2, name="rs_in")
                rs_out = dram.tile([128, D], bf16, tag="rsout", bufs=2, name="rs_out")
                for mh in range(4):
                    ps = {}
                    for tc2 in range(2):
                        for mc2 in range(2):
                            ps[(tc2, mc2)] = pspool.tile(
                                [128, 512], fp32,
                                tag=f"b{(mh % 2) * 4 + tc2 * 2 + mc2}",
                                name=f"pso{tc2}{mc2}",
                            )
                    for cc in range(HG):
                        wt = wvpool.tile(
                            [128, 1024], bf16, tag="wo", bufs=4, name="wot"
                        )
                        nc.sync.dma_start(
                            wt[:],
                            wo[cc * 128 : (cc + 1) * 128,
                               mh * 1024 : (mh + 1) * 1024],
                        )
                        for tc2 in range(2):
                            t128 = tq * 2 + tc2
                            for mc2 in range(2):
                                nc.tensor.matmul(
                                    ps[(tc2, mc2)][:],
                                    oT_sb[:, cc, t128 * 128 : (t128 + 1) * 128],
                                    wt[:, mc2 * 512 : (mc2 + 1) * 512],
                                    start=(cc == 0),
                                    stop=(cc == HG - 1),
                                )
                    for tc2 in range(2):
                        po = outpool.tile([128, 1024], bf16, tag="po", name="po")
                        for mc2 in range(2):
                            m0 = mh * 1024 + mc2 * 512
                            nc.vector.tensor_add(
                                po[:, mc2 * 512 : (mc2 + 1) * 512],
                                ps[(tc2, mc2)][:],
                                bo_sb[:, m0 : m0 + 512],
                            )
                        nc.gpsimd.dma_start(
                            rs_in[tc2 * 128 : (tc2 + 1) * 128,
                                  mh * 1024 : (mh + 1) * 1024],
                            po[:],
                        )
                nc.gpsimd.collective_compute(
                    "ReduceScatter",
                    mybir.AluOpType.add,
                    replica_groups=RG,
                    ins=[rs_in[:].opt()],
                    outs=[rs_out[:].opt()],
                )
                for dh in range(4):
                    fin_bf = evacpool.tile(
                        [128, 1024], bf16, tag="finbf", name="fin_bf"
                    )
                    fin_f32 = evacpool.tile(
                        [128, 1024], fp32, tag="finf32", name="fin_f32"
                    )
                    nc.scalar.dma_start(
                        fin_bf[:], rs_out[:, dh * 1024 : (dh + 1) * 1024]
                    )
                    nc.vector.tensor_copy(fin_f32[:], fin_bf[:])
                    nc.scalar.dma_start(
                        out[tq, :, dh * 1024 : (dh + 1) * 1024], fin_f32[:]
                    )

    nc.finalize()
    return nc


def _prep_shards(x, attn_bias, wq_kernel, wq_bias, wk_kernel, wk_bias,
                 wv_kernel, wv_bias, wo_kernel, wo_bias):
    """Host-side shard prep. Returns in_maps (list of 8 dicts)."""
    freqs = 1.0 / 10000.0 ** (np.arange(0, ROTARY, 2) / ROTARY)  # [16]
    pos = np.arange(MAX_POS - S, MAX_POS)  # [S]
    ang = np.outer(freqs, pos)  # [16, S]
    rotC = np.cos(ang).astype(np.float32)
    rotS = np.sin(ang).astype(np.float32)
    ones = np.ones((128, 1), dtype=BF16)
    biasT = np.ascontiguousarray(attn_bias[0, 0].T).astype(BF16)

    in_maps = []
    for c in range(NCORES):
        b, g = c // 2, c % 2
        hs = slice(g * HG, (g + 1) * HG)
        in_maps.append(
            {
                "xT": np.ascontiguousarray(x[b].T).astype(BF16),
                "wq": (wq_kernel[:, hs, :].reshape(D, F) * SCALE).astype(BF16),
                "wk": wk_kernel[:, hs, :].reshape(D, F).astype(BF16),
                "wv": wv_kernel[:, hs, :].reshape(D, F).astype(BF16),
                "wo": wo_kernel[hs].reshape(F, D).astype(BF16),
                "bqT": np.ascontiguousarray((wq_bias[hs] * SCALE).T).astype(
                    np.float32
                ),
                "bkT": np.ascontiguousarray(wk_bias[hs].T).astype(np.float32),
                "bv_bc": np.broadcast_to(
                    wv_bias[hs].reshape(1, F), (128, F)
                ).astype(BF16).copy(),
                "bo_bc": np.broadcast_to(
                    (wo_bias * 0.5).reshape(1, D), (128, D)
                ).astype(BF16).copy(),
                "biasT": biasT,
                "rotC": rotC.astype(BF16),
                "rotS": rotS.astype(BF16),
                "ones": ones,
                "ones_row": np.ones((1, 128), dtype=np.float32),
            }
        )
    return in_maps


def kernel(x, attn_bias, wq_kernel, wq_bias, wk_kernel, wk_bias,
           wv_kernel, wv_bias, wo_kernel, wo_bias, _trace=False):
    from concourse import bass_utils

    if "nc" not in _cache:
        _cache["nc"] = _build()
    nc = _cache["nc"]

    in_maps = _prep_shards(
        np.asarray(x), np.asarray(attn_bias),
        np.asarray(wq_kernel), np.asarray(wq_bias),
        np.asarray(wk_kernel), np.asarray(wk_bias),
        np.asarray(wv_kernel), np.asarray(wv_bias),
        np.asarray(wo_kernel), np.asarray(wo_bias),
    )
    res = bass_utils.run_bass_kernel_spmd(
        nc, in_maps, core_ids=list(range(NCORES)), trace=_trace
    )
    _cache["last_results"] = res

    full = np.empty((B, S, D), dtype=np.float32)
    for b in range(B):
        lo = res.results[2 * b]["out"]  # [4, 128, D]: tokens tq*256 .. +128
        hi = res.results[2 * b + 1]["out"]  # tokens tq*256+128 .. +256
        for tq in range(4):
            full[b, tq * 256 : tq * 256 + 128] = lo[tq]
            full[b, tq * 256 + 128 : (tq + 1) * 256] = hi[tq]
    return full


# revision 28
# speedup vs baseline: 1.3958x; 1.0660x over previous
"""Distributed Bass kernel for attention (B=4,S=1024,D=4096,H=32,HD=128).

Sharding: 8 cores = 4 batch x 2 head-groups of 16 heads (core c: batch c//2,
heads (c%2)*16..+16). Per-core pipeline (all matmuls bf16 with fp32 PSUM):

  1. QKV projections. q/k computed feature-major (q^T[hd,tok] per head) so
     attention needs no transposes; v computed token-major. x and weights are
     streamed; q/k get partial rotary applied in-SBUF then bounce via DRAM.
     Rotary uses the block-order trick: scores are invariant to a channel
     permutation applied identically to q and k, so the rotated (real,imag)
     halves stay block-contiguous instead of interleaved.
  2. Scores computed directly transposed E[k,q] = exp(kT.T @ qT + biasT).
     No max subtraction (scores bounded for this input distribution); the
     softmax denominator is a ones-vector matmul reducing over partitions.
     1/s is folded into the o^T PSUM evacuation (o^T = v.T @ E unnormalized).
  3. Output projection partial sums with wo_bias/2 folded in, chunked
     pairwise ReduceScatter (bf16) over token quarters, fp32 cast, DMA out.
"""

import sys

sys.path.insert(0, "/opt/trn_rl_repo")

import numpy as np
import ml_dtypes

BF16 = ml_dtypes.bfloat16

B, S, D, H, HD = 4, 1024, 4096, 32, 128
ROTARY = 32
MAX_POS = 10000
HG = H // 2  # heads per core = 16
F = HG * HD  # per-core qkv feature dim = 2048
NCORES = 8
SCALE = 1.0 / np.sqrt(HD)
NDC = D // 128  # 32 contraction chunks
NKC = S // 128  # 8 key chunks

_cache = {}


def _build():
    import concourse.mybir as mybir
    import concourse.tile as tile
    from concourse import bacc

    fp32 = mybir.dt.float32
    bf16 = mybir.dt.bfloat16
    Act = mybir.ActivationFunctionType

    nc = bacc.Bacc("TRN2", target_bir_lowering=False, num_devices=NCORES)

    # ---- DRAM parameters (per-core shards) ----
    xT = nc.dram_tensor("xT", [D, S], bf16, kind="ExternalInput")
    wq = nc.dram_tensor("wq", [D, F], bf16, kind="ExternalInput")
    wk = nc.dram_tensor("wk", [D, F], bf16, kind="ExternalInput")
    wv = nc.dram_tensor("wv", [D, F], bf16, kind="ExternalInput")
    wo = nc.dram_tensor("wo", [F, D], bf16, kind="ExternalInput")
    bqT = nc.dram_tensor("bqT", [HD, HG], fp32, kind="ExternalInput")
    bkT = nc.dram_tensor("bkT", [HD, HG], fp32, kind="ExternalInput")
    bv_bc = nc.dram_tensor("bv_bc", [128, F], bf16, kind="ExternalInput")
    bo_bc = nc.dram_tensor("bo_bc", [128, D], bf16, kind="ExternalInput")
    biasT = nc.dram_tensor("biasT", [S, S], bf16, kind="ExternalInput")
    rotC = nc.dram_tensor("rotC", [16, S], bf16, kind="ExternalInput")
    rotS = nc.dram_tensor("rotS", [16, S], bf16, kind="ExternalInput")
    ones = nc.dram_tensor("ones", [128, 1], bf16, kind="ExternalInput")
    ones_row = nc.dram_tensor("ones_row", [1, 128], fp32, kind="ExternalInput")
    out = nc.dram_tensor("out", [4, 4, 128, 1024], fp32, kind="ExternalOutput")

    RG = [[0, 1], [2, 3], [4, 5], [6, 7]]

    with tile.TileContext(nc) as tc:
        with (
            tc.tile_pool(name="wpool", bufs=3) as wpool,
            tc.tile_pool(name="wvpool", bufs=2) as wvpool,
            tc.tile_pool(name="xpool", bufs=2) as xpool,
            tc.tile_pool(name="stream", bufs=2) as stream,
            tc.tile_pool(name="stage", bufs=2) as stpool,
            tc.tile_pool(name="tmp", bufs=1) as tmppool,
            tc.tile_pool(name="small", bufs=1) as small,
            tc.tile_pool(name="epool", bufs=1) as epool,
            tc.tile_pool(name="big", bufs=1) as big,
            tc.tile_pool(name="evac", bufs=2) as evacpool,
            tc.tile_pool(name="outp", bufs=2) as outpool,
            tc.tile_pool(name="ps", bufs=1, space="PSUM") as pspool,
            tc.tile_pool(name="dram", bufs=1, space="DRAM") as dram,
        ):
            # ---- constants ----
            bqT_sb = small.tile([HD, HG], fp32)
            bkT_sb = small.tile([HD, HG], fp32)
            bv_sb = small.tile([128, F], bf16)
            bo_sb = small.tile([128, D], bf16)
            rotC_sb = small.tile([16, S], bf16)
            rotS_sb = small.tile([16, S], bf16)
            ones_sb = small.tile([128, 1], bf16)
            ones_row_sb = small.tile([1, 128], fp32)
            nc.sync.dma_start(ones_row_sb[:], ones_row[:])
            nc.sync.dma_start(bqT_sb[:], bqT[:])
            nc.sync.dma_start(bkT_sb[:], bkT[:])
            nc.sync.dma_start(bv_sb[:], bv_bc[:])
            nc.sync.dma_start(bo_sb[:], bo_bc[:])
            nc.sync.dma_start(rotC_sb[:], rotC[:])
            nc.sync.dma_start(rotS_sb[:], rotS[:])
            nc.sync.dma_start(ones_sb[:], ones[:])

            # resident input activations [d, dc, tok] (64KB/part);
            # oT_sb later reuses this slot (x is dead after the V pass)
            xT_sb = big.tile([128, NDC, S], bf16, tag="bigbuf", name="xT_sb")
            for i in range(4):
                nc.sync.dma_start(
                    xT_sb[:, i * 8 : (i + 1) * 8, :],
                    xT[i * 1024 : (i + 1) * 1024, :].rearrange(
                        "(a p) t -> p a t", p=128
                    ),
                )

            # DRAM bounce tensors
            q_dram = dram.tile([HG, 128, S], bf16, name="q_dram")
            k_dram = dram.tile([HG, 128, S], bf16, name="k_dram")
            v_dram = dram.tile([NKC, 128, F], bf16, name="v_dram")

            # ================= Phase 1: QKV projections =================
            def qk_pass(w_dram, bias_sb, dst_dram, which):
                for hg4 in range(4):
                    ps = {}
                    for hi in range(4):
                        for th in range(2):
                            ps[(hi, th)] = pspool.tile(
                                [128, 512], fp32, tag=f"b{hi * 2 + th}",
                                name=f"ps{which}{hi}{th}",
                            )
                    for dc in range(NDC):
                        wt = wpool.tile([128, 512], bf16, tag=f"w{which}", name="wt")
                        nc.sync.dma_start(
                            wt[:],
                            w_dram[dc * 128 : (dc + 1) * 128,
                                   hg4 * 512 : (hg4 + 1) * 512],
                        )
                        for hi in range(4):
                            for th in range(2):
                                nc.tensor.matmul(
                                    ps[(hi, th)][:],
                                    wt[:, hi * 128 : (hi + 1) * 128],
                                    xT_sb[:, dc, th * 512 : (th + 1) * 512],
                                    start=(dc == 0),
                                    stop=(dc == NDC - 1),
                                )
                    for hi in range(4):
                        h = hg4 * 4 + hi
                        qbf = stpool.tile(
                            [128, S], bf16, tag="qbf", bufs=5, name="qbf"
                        )
                        for th in range(2):
                            # single full-tile evac+bias to bf16 on ACT
                            # (PSUM released by this one op; rotary reads
                            # bf16 from qbf and overwrites [0:32] in place)
                            nc.scalar.activation(
                                qbf[:, th * 512 : (th + 1) * 512],
                                ps[(hi, th)][:],
                                Act.Identity,
                                bias=bias_sb[:, h : h + 1],
                            )
                        # rotary (block order): u=qbf[0:16], w=qbf[16:32].
                        # Engine ops need 32-aligned partition bases, so the
                        # w half bounces via DMA to a base-0 tile and the f
                        # result bounces back to partitions 16:32.
                        rot_w = tmppool.tile([16, S], bf16, tag="rw", name="rot_w")
                        nc.scalar.dma_start(rot_w[:], qbf[16:32, :])
                        t1 = tmppool.tile([16, S], bf16, tag="t1", name="t1")
                        t2 = tmppool.tile([16, S], bf16, tag="t2", name="t2")
                        t3 = tmppool.tile([16, S], bf16, tag="t3", name="t3")
                        t4 = tmppool.tile([16, S], bf16, tag="t4", name="t4")
                        fbuf = tmppool.tile([16, S], bf16, tag="fb", name="fbuf")
                        u = qbf[0:16, :]
                        nc.vector.tensor_mul(t1[:], u, rotC_sb[:])
                        nc.vector.tensor_mul(t3[:], rot_w[:], rotS_sb[:])
                        nc.gpsimd.tensor_mul(t2[:], u, rotS_sb[:])
                        nc.gpsimd.tensor_mul(t4[:], rot_w[:], rotC_sb[:])
                        nc.vector.tensor_sub(qbf[0:16, :], t1[:], t3[:])
                        nc.gpsimd.tensor_add(fbuf[:], t2[:], t4[:])
                        nc.gpsimd.dma_start(qbf[16:32, :], fbuf[:])
                        nc.gpsimd.dma_start(dst_dram[h], qbf[:])

            qk_pass(wq, bqT_sb, q_dram, "q")
            qk_pass(wk, bkT_sb, k_dram, "k")

            # V pass (token-major): psum[tok=128, feat=512]
            for fh in range(2):
                for tq4 in range(2):
                    ps = {}
                    for ti in range(4):
                        for fc in range(2):
                            ps[(ti, fc)] = pspool.tile(
                                [128, 512], fp32, tag=f"b{ti * 2 + fc}",
                                name=f"psv{ti}{fc}",
                            )
                    for dc in range(NDC):
                        wt = wvpool.tile([128, 1024], bf16, tag="wv", bufs=3, name="wvt")
                        nc.sync.dma_start(
                            wt[:],
                            wv[dc * 128 : (dc + 1) * 128,
                               fh * 1024 : (fh + 1) * 1024],
                        )
                        for ti in range(4):
                            tci = tq4 * 4 + ti
                            for fc in range(2):
                                nc.tensor.matmul(
                                    ps[(ti, fc)][:],
                                    xT_sb[:, dc, tci * 128 : (tci + 1) * 128],
                                    wt[:, fc * 512 : (fc + 1) * 512],
                                    start=(dc == 0),
                                    stop=(dc == NDC - 1),
                                )
                    for ti in range(4):
                        tci = tq4 * 4 + ti
                        for fc in range(2):
                            f0 = fh * 1024 + fc * 512
                            vb = evacpool.tile([128, 512], bf16, tag="vb", name="vb")
                            nc.vector.tensor_add(
                                vb[:], ps[(ti, fc)][:], bv_sb[:, f0 : f0 + 512]
                            )
                            nc.scalar.dma_start(v_dram[tci, :, f0 : f0 + 512], vb[:])

            # ================= Phase 2: attention per head =================
            oT_sb = big.tile(
                [128, HG, S], bf16, tag="bigbuf", name="oT_sb"
            )  # [hd, h, tok]
            for h in range(HG):
                qh_t = stream.tile([128, S], bf16, tag="qh", name="qh_t")
                kh_t = stream.tile([128, S], bf16, tag="kh", name="kh_t")
                vh_t = stream.tile([128, NKC, HD], bf16, tag="vh", name="vh_t")
                nc.sync.dma_start(qh_t[:], q_dram[h])
                nc.sync.dma_start(kh_t[:], k_dram[h])
                nc.sync.dma_start(
                    vh_t[:],
                    v_dram[:, :, h * 128 : (h + 1) * 128].rearrange(
                        "kc p hd -> p kc hd"
                    ),
                )
                E = epool.tile([128, NKC, S], bf16, tag="E", bufs=2, name="E")
                sum_ps = {}
                o_ps = {}
                for qh in range(2):
                    sum_ps[qh] = pspool.tile(
                        [1, 512], fp32, tag=f"b{2 + qh}", name=f"sum{qh}"
                    )
                    o_ps[qh] = pspool.tile(
                        [128, 512], fp32, tag=f"b{4 + qh}", name=f"o{qh}"
                    )
                for kc in range(NKC):
                    bt = stream.tile([128, S], bf16, tag="bt", name="bt")
                    nc.sync.dma_start(bt[:], biasT[kc * 128 : (kc + 1) * 128, :])
                    for qh in range(2):
                        qs = slice(qh * 512, (qh + 1) * 512)
                        stag = (0, 1, 6, 7)[(kc * 2 + qh) % 4]
                        sps = pspool.tile(
                            [128, 512], fp32, tag=f"b{stag}", name="sps"
                        )
                        nc.tensor.matmul(
                            sps[:],
                            kh_t[:, kc * 128 : (kc + 1) * 128],
                            qh_t[:, qs],
                            start=True,
                            stop=True,
                        )
                        nc.vector.tensor_add(E[:, kc, qs], sps[:], bt[:, qs])
                        nc.scalar.activation(
                            E[:, kc, qs], E[:, kc, qs], Act.Exp
                        )
                        # sum and o both consume E chunk-by-chunk, so the
                        # PE never waits for the whole row of E
                        nc.tensor.matmul(
                            sum_ps[qh][:],
                            ones_sb[:],
                            E[:, kc, qs],
                            start=(kc == 0),
                            stop=(kc == NKC - 1),
                        )
                        nc.tensor.matmul(
                            o_ps[qh][:],
                            vh_t[:, kc, :],
                            E[:, kc, qs],
                            start=(kc == 0),
                            stop=(kc == NKC - 1),
                        )
                inv_row = tmppool.tile(
                    [1, S], fp32, tag="inv", bufs=2, name="inv_row"
                )
                inv_bc = tmppool.tile(
                    [128, S], fp32, tag="invbc", bufs=1, name="inv_bc"
                )
                for qh in range(2):
                    nc.vector.reciprocal_approx_fast(
                        inv_row[:, qh * 512 : (qh + 1) * 512], sum_ps[qh][:]
                    )
                for qh in range(2):
                    qs = slice(qh * 512, (qh + 1) * 512)
                    # rank-1 PE broadcast: ones[128] x inv_row -> [128, 512]
                    bc_ps = pspool.tile(
                        [128, 512], fp32, tag=f"b{2 + qh}", name="bc_ps"
                    )
                    nc.tensor.matmul(
                        bc_ps[:],
                        ones_row_sb[:],
                        inv_row[:, qs],
                        start=True,
                        stop=True,
                    )
                    nc.vector.tensor_copy(inv_bc[:, qs], bc_ps[:])
                    nc.vector.tensor_mul(
                        oT_sb[:, h, qs], o_ps[qh][:], inv_bc[:, qs]
                    )

            # ========= Phase 3: out-projection + chunked ReduceScatter =========
            # chunk by m-quarter (1024 cols): wo is read only twice (once per
            # t128-group of 4), and each m-quarter ReduceScatters [1024,1024]
            # bf16 over the pair while the next quarter computes.
            for mh4 in range(4):
                rs_in = dram.tile([S, 1024], bf16, tag="rsin", bufs=2, name="rs_in")
                rs_out = dram.tile(
                    [512, 1024], bf16, tag="rsout", bufs=2, name="rs_out"
                )
                for tg in range(2):
                    ps = {}
                    for ti in range(4):
                        for mc2 in range(2):
                            ps[(ti, mc2)] = pspool.tile(
                                [128, 512], fp32,
                                tag=f"b{ti * 2 + mc2}",
                                name=f"pso{ti}{mc2}",
                            )
                    for cc in range(HG):
                        wt = wvpool.tile(
                            [128, 1024], bf16, tag="wo", bufs=4, name="wot"
                        )
                        nc.sync.dma_start(
                            wt[:],
                            wo[cc * 128 : (cc + 1) * 128,
                               mh4 * 1024 : (mh4 + 1) * 1024],
                        )
                        for ti in range(4):
                            t128 = tg * 4 + ti
                            for mc2 in range(2):
                                nc.tensor.matmul(
                                    ps[(ti, mc2)][:],
                                    oT_sb[:, cc, t128 * 128 : (t128 + 1) * 128],
                                    wt[:, mc2 * 512 : (mc2 + 1) * 512],
                                    start=(cc == 0),
                                    stop=(cc == HG - 1),
                                )
                    for ti in range(4):
                        t128 = tg * 4 + ti
                        po = outpool.tile([128, 1024], bf16, tag="po", name="po")
                        for mc2 in range(2):
                            m0 = mh4 * 1024 + mc2 * 512
                            nc.vector.tensor_add(
                                po[:, mc2 * 512 : (mc2 + 1) * 512],
                                ps[(ti, mc2)][:],
                                bo_sb[:, m0 : m0 + 512],
                            )
                        nc.gpsimd.dma_start(
                            rs_in[t128 * 128 : (t128 + 1) * 128, :], po[:]
                        )
                nc.gpsimd.collective_compute(
                    "ReduceScatter",
                    mybir.AluOpType.add,
                    replica_groups=RG,
                    ins=[rs_in[:].opt()],
                    outs=[rs_out[:].opt()],
                )
                for dh in range(4):
                    fin_bf = evacpool.tile(
                        [128, 1024], bf16, tag="finbf", name="fin_bf"
                    )
                    fin_f32 = evacpool.tile(
                        [128, 1024], fp32, tag="finf32", name="fin_f32"
                    )
                    nc.scalar.dma_start(
                        fin_bf[:], rs_out[dh * 128 : (dh + 1) * 128, :]
                    )
                    nc.vector.tensor_copy(fin_f32[:], fin_bf[:])
                    nc.scalar.dma_start(out[mh4, dh, :, :], fin_f32[:])

    nc.finalize()
    return nc


def _prep_shards(x, attn_bias, wq_kernel, wq_bias, wk_kernel, wk_bias,
                 wv_kernel, wv_bias, wo_kernel, wo_bias):
    """Host-side shard prep. Returns in_maps (list of 8 dicts)."""
    freqs = 1.0 / 10000.0 ** (np.arange(0, ROTARY, 2) / ROTARY)  # [16]
    pos = np.arange(MAX_POS - S, MAX_POS)  # [S]
    ang = np.outer(freqs, pos)  # [16, S]
    rotC = np.cos(ang).astype(np.float32)
    rotS = np.sin(ang).astype(np.float32)
    ones = np.ones((128, 1), dtype=BF16)
    biasT = np.ascontiguousarray(attn_bias[0, 0].T).astype(BF16)

    in_maps = []
    for c in range(NCORES):
        b, g = c // 2, c % 2
        hs = slice(g * HG, (g + 1) * HG)
        in_maps.append(
            {
                "xT": np.ascontiguousarray(x[b].T).astype(BF16),
                "wq": (wq_kernel[:, hs, :].reshape(D, F) * SCALE).astype(BF16),
                "wk": wk_kernel[:, hs, :].reshape(D, F).astype(BF16),
                "wv": wv_kernel[:, hs, :].reshape(D, F).astype(BF16),
                "wo": wo_kernel[hs].reshape(F, D).astype(BF16),
                "bqT": np.ascontiguousarray((wq_bias[hs] * SCALE).T).astype(
                    np.float32
                ),
                "bkT": np.ascontiguousarray(wk_bias[hs].T).astype(np.float32),
                "bv_bc": np.broadcast_to(
                    wv_bias[hs].reshape(1, F), (128, F)
                ).astype(BF16).copy(),
                "bo_bc": np.broadcast_to(
                    (wo_bias * 0.5).reshape(1, D), (128, D)
                ).astype(BF16).copy(),
                "biasT": biasT,
                "rotC": rotC.astype(BF16),
                "rotS": rotS.astype(BF16),
                "ones": ones,
                "ones_row": np.ones((1, 128), dtype=np.float32),
            }
        )
    return in_maps


def kernel(x, attn_bias, wq_kernel, wq_bias, wk_kernel, wk_bias,
           wv_kernel, wv_bias, wo_kernel, wo_bias, _trace=False):
    from concourse import bass_utils

    if "nc" not in _cache:
        _cache["nc"] = _build()
    nc = _cache["nc"]

    in_maps = _prep_shards(
        np.asarray(x), np.asarray(attn_bias),
        np.asarray(wq_kernel), np.asarray(wq_bias),
        np.asarray(wk_kernel), np.asarray(wk_bias),
        np.asarray(wv_kernel), np.asarray(wv_bias),
        np.asarray(wo_kernel), np.asarray(wo_bias),
    )
    res = bass_utils.run_bass_kernel_spmd(
        nc, in_maps, core_ids=list(range(NCORES)), trace=_trace
    )
    _cache["last_results"] = res

    full = np.empty((B, S, D), dtype=np.float32)
    for b in range(B):
        lo = res.results[2 * b]["out"]  # [4 mh4, 4, 128, 1024]: tokens 0:512
        hi = res.results[2 * b + 1]["out"]  # tokens 512:1024
        for mh4 in range(4):
            ms = slice(mh4 * 1024, (mh4 + 1) * 1024)
            full[b, 0:512, ms] = lo[mh4].reshape(512, 1024)
            full[b, 512:1024, ms] = hi[mh4].reshape(512, 1024)
    return full


# revision 29
# speedup vs baseline: 1.4218x; 1.0187x over previous
"""Distributed Bass kernel for attention (B=4,S=1024,D=4096,H=32,HD=128).

Sharding: 8 cores = 4 batch x 2 head-groups of 16 heads (core c: batch c//2,
heads (c%2)*16..+16). Per-core pipeline (all matmuls bf16 with fp32 PSUM):

  1. QKV projections. q/k computed feature-major (q^T[hd,tok] per head) so
     attention needs no transposes; v computed token-major. x and weights are
     streamed; q/k get partial rotary applied in-SBUF then bounce via DRAM.
     Rotary uses the block-order trick: scores are invariant to a channel
     permutation applied identically to q and k, so the rotated (real,imag)
     halves stay block-contiguous instead of interleaved.
  2. Scores computed directly transposed E[k,q] = exp(kT.T @ qT + biasT).
     No max subtraction (scores bounded for this input distribution); the
     softmax denominator is a ones-vector matmul reducing over partitions.
     1/s is folded into the o^T PSUM evacuation (o^T = v.T @ E unnormalized).
  3. Output projection partial sums with wo_bias/2 folded in, chunked
     pairwise ReduceScatter (bf16) over token quarters, fp32 cast, DMA out.
"""

import sys

sys.path.insert(0, "/opt/trn_rl_repo")

import numpy as np
import ml_dtypes

BF16 = ml_dtypes.bfloat16

B, S, D, H, HD = 4, 1024, 4096, 32, 128
ROTARY = 32
MAX_POS = 10000
HG = H // 2  # heads per core = 16
F = HG * HD  # per-core qkv feature dim = 2048
NCORES = 8
SCALE = 1.0 / np.sqrt(HD)
NDC = D // 128  # 32 contraction chunks
NKC = S // 128  # 8 key chunks

_cache = {}


def _build():
    import concourse.mybir as mybir
    import concourse.tile as tile
    from concourse import bacc

    fp32 = mybir.dt.float32
    bf16 = mybir.dt.bfloat16
    Act = mybir.ActivationFunctionType

    nc = bacc.Bacc("TRN2", target_bir_lowering=False, num_devices=NCORES)

    # ---- DRAM parameters (per-core shards) ----
    xT = nc.dram_tensor("xT", [D, S], bf16, kind="ExternalInput")
    wq = nc.dram_tensor("wq", [D, F], bf16, kind="ExternalInput")
    wk = nc.dram_tensor("wk", [D, F], bf16, kind="ExternalInput")
    wv = nc.dram_tensor("wv", [D, F], bf16, kind="ExternalInput")
    wo = nc.dram_tensor("wo", [F, D], bf16, kind="ExternalInput")
    bqT = nc.dram_tensor("bqT", [HD, HG], fp32, kind="ExternalInput")
    bkT = nc.dram_tensor("bkT", [HD, HG], fp32, kind="ExternalInput")
    bv_bc = nc.dram_tensor("bv_bc", [128, F], bf16, kind="ExternalInput")
    bo_bc = nc.dram_tensor("bo_bc", [128, D], bf16, kind="ExternalInput")
    biasT = nc.dram_tensor("biasT", [S, S], bf16, kind="ExternalInput")
    rotC = nc.dram_tensor("rotC", [16, S], bf16, kind="ExternalInput")
    rotS = nc.dram_tensor("rotS", [16, S], bf16, kind="ExternalInput")
    ones = nc.dram_tensor("ones", [128, 1], bf16, kind="ExternalInput")
    ones_row = nc.dram_tensor("ones_row", [1, 128], fp32, kind="ExternalInput")
    ones_row_bf = nc.dram_tensor("ones_row_bf", [1, 128], bf16, kind="ExternalInput")
    out = nc.dram_tensor("out", [4, 2, 2, 128, 1024], fp32, kind="ExternalOutput")

    RG = [[0, 1], [2, 3], [4, 5], [6, 7]]

    with tile.TileContext(nc) as tc:
        with (
            tc.tile_pool(name="wpool", bufs=3) as wpool,
            tc.tile_pool(name="wvpool", bufs=2) as wvpool,
            tc.tile_pool(name="xpool", bufs=2) as xpool,
            tc.tile_pool(name="stream", bufs=2) as stream,
            tc.tile_pool(name="stage", bufs=2) as stpool,
            tc.tile_pool(name="tmp", bufs=1) as tmppool,
            tc.tile_pool(name="small", bufs=1) as small,
            tc.tile_pool(name="epool", bufs=1) as epool,
            tc.tile_pool(name="big", bufs=1) as big,
            tc.tile_pool(name="evac", bufs=2) as evacpool,
            tc.tile_pool(name="outp", bufs=2) as outpool,
            tc.tile_pool(name="ps", bufs=1, space="PSUM") as pspool,
            tc.tile_pool(name="dram", bufs=1, space="DRAM") as dram,
        ):
            # ---- constants ----
            bqT_sb = small.tile([HD, HG], fp32)
            bkT_sb = small.tile([HD, HG], fp32)
            bv_sb = small.tile([128, F], bf16)
            bo_sb = small.tile([128, D], bf16)
            rotC_sb = small.tile([16, S], bf16)
            rotS_sb = small.tile([16, S], bf16)
            ones_sb = small.tile([128, 1], bf16)
            ones_row_sb = small.tile([1, 128], fp32)
            ones_row_bf_sb = small.tile([1, 128], bf16)
            nc.sync.dma_start(ones_row_sb[:], ones_row[:])
            nc.sync.dma_start(ones_row_bf_sb[:], ones_row_bf[:])
            nc.sync.dma_start(bqT_sb[:], bqT[:])
            nc.sync.dma_start(bkT_sb[:], bkT[:])
            nc.sync.dma_start(bv_sb[:], bv_bc[:])
            nc.sync.dma_start(bo_sb[:], bo_bc[:])
            nc.sync.dma_start(rotC_sb[:], rotC[:])
            nc.sync.dma_start(rotS_sb[:], rotS[:])
            nc.sync.dma_start(ones_sb[:], ones[:])

            # resident input activations [d, dc, tok] (64KB/part);
            # oT_sb later reuses this slot (x is dead after the V pass)
            xT_sb = big.tile([128, NDC, S], bf16, tag="bigbuf", name="xT_sb")
            for i in range(4):
                nc.sync.dma_start(
                    xT_sb[:, i * 8 : (i + 1) * 8, :],
                    xT[i * 1024 : (i + 1) * 1024, :].rearrange(
                        "(a p) t -> p a t", p=128
                    ),
                )

            # DRAM bounce tensors
            q_dram = dram.tile([HG, 128, S], bf16, name="q_dram")
            k_dram = dram.tile([HG, 128, S], bf16, name="k_dram")
            v_dram = dram.tile([NKC, 128, F], bf16, name="v_dram")

            # ================= Phase 1: QKV projections =================
            def qk_pass(w_dram, bias_sb, dst_dram, which):
                for hg4 in range(4):
                    ps = {}
                    for hi in range(4):
                        for th in range(2):
                            ps[(hi, th)] = pspool.tile(
                                [128, 512], fp32, tag=f"b{hi * 2 + th}",
                                name=f"ps{which}{hi}{th}",
                            )
                    for dc in range(NDC):
                        wt = wpool.tile([128, 512], bf16, tag=f"w{which}", name="wt")
                        nc.sync.dma_start(
                            wt[:],
                            w_dram[dc * 128 : (dc + 1) * 128,
                                   hg4 * 512 : (hg4 + 1) * 512],
                        )
                        for hi in range(4):
                            for th in range(2):
                                nc.tensor.matmul(
                                    ps[(hi, th)][:],
                                    wt[:, hi * 128 : (hi + 1) * 128],
                                    xT_sb[:, dc, th * 512 : (th + 1) * 512],
                                    start=(dc == 0),
                                    stop=(dc == NDC - 1),
                                )
                    for hi in range(4):
                        h = hg4 * 4 + hi
                        qbf = stpool.tile(
                            [128, S], bf16, tag="qbf", bufs=5, name="qbf"
                        )
                        for th in range(2):
                            # single full-tile evac+bias to bf16 on ACT
                            # (PSUM released by this one op; rotary reads
                            # bf16 from qbf and overwrites [0:32] in place)
                            nc.scalar.activation(
                                qbf[:, th * 512 : (th + 1) * 512],
                                ps[(hi, th)][:],
                                Act.Identity,
                                bias=bias_sb[:, h : h + 1],
                            )
                        # rotary (block order): u=qbf[0:16], w=qbf[16:32].
                        # Engine ops need 32-aligned partition bases, so the
                        # w half bounces via DMA to a base-0 tile and the f
                        # result bounces back to partitions 16:32.
                        rot_w = tmppool.tile([16, S], bf16, tag="rw", name="rot_w")
                        nc.scalar.dma_start(rot_w[:], qbf[16:32, :])
                        t1 = tmppool.tile([16, S], bf16, tag="t1", name="t1")
                        t2 = tmppool.tile([16, S], bf16, tag="t2", name="t2")
                        t3 = tmppool.tile([16, S], bf16, tag="t3", name="t3")
                        t4 = tmppool.tile([16, S], bf16, tag="t4", name="t4")
                        fbuf = tmppool.tile([16, S], bf16, tag="fb", name="fbuf")
                        u = qbf[0:16, :]
                        nc.vector.tensor_mul(t1[:], u, rotC_sb[:])
                        nc.vector.tensor_mul(t3[:], rot_w[:], rotS_sb[:])
                        nc.gpsimd.tensor_mul(t2[:], u, rotS_sb[:])
                        nc.gpsimd.tensor_mul(t4[:], rot_w[:], rotC_sb[:])
                        nc.vector.tensor_sub(qbf[0:16, :], t1[:], t3[:])
                        nc.gpsimd.tensor_add(fbuf[:], t2[:], t4[:])
                        nc.gpsimd.dma_start(qbf[16:32, :], fbuf[:])
                        nc.gpsimd.dma_start(dst_dram[h], qbf[:])

            qk_pass(wq, bqT_sb, q_dram, "q")
            qk_pass(wk, bkT_sb, k_dram, "k")

            # V pass (token-major): psum[tok=128, feat=512]
            for fh in range(2):
                for tq4 in range(2):
                    ps = {}
                    for ti in range(4):
                        for fc in range(2):
                            ps[(ti, fc)] = pspool.tile(
                                [128, 512], fp32, tag=f"b{ti * 2 + fc}",
                                name=f"psv{ti}{fc}",
                            )
                    for dc in range(NDC):
                        wt = wvpool.tile([128, 1024], bf16, tag="wv", bufs=3, name="wvt")
                        nc.sync.dma_start(
                            wt[:],
                            wv[dc * 128 : (dc + 1) * 128,
                               fh * 1024 : (fh + 1) * 1024],
                        )
                        for ti in range(4):
                            tci = tq4 * 4 + ti
                            for fc in range(2):
                                nc.tensor.matmul(
                                    ps[(ti, fc)][:],
                                    xT_sb[:, dc, tci * 128 : (tci + 1) * 128],
                                    wt[:, fc * 512 : (fc + 1) * 512],
                                    start=(dc == 0),
                                    stop=(dc == NDC - 1),
                                )
                    for ti in range(4):
                        tci = tq4 * 4 + ti
                        for fc in range(2):
                            f0 = fh * 1024 + fc * 512
                            vb = evacpool.tile([128, 512], bf16, tag="vb", name="vb")
                            nc.vector.tensor_add(
                                vb[:], ps[(ti, fc)][:], bv_sb[:, f0 : f0 + 512]
                            )
                            nc.scalar.dma_start(v_dram[tci, :, f0 : f0 + 512], vb[:])

            # ================= Phase 2: attention per head =================
            oT_sb = big.tile(
                [128, HG, S], bf16, tag="bigbuf", name="oT_sb"
            )  # [hd, h, tok]
            for h in range(HG):
                qh_t = stream.tile([128, S], bf16, tag="qh", name="qh_t")
                kh_t = stream.tile([128, S], bf16, tag="kh", name="kh_t")
                vh_t = stream.tile([128, NKC, HD], bf16, tag="vh", name="vh_t")
                nc.sync.dma_start(qh_t[:], q_dram[h])
                nc.sync.dma_start(kh_t[:], k_dram[h])
                nc.sync.dma_start(
                    vh_t[:],
                    v_dram[:, :, h * 128 : (h + 1) * 128].rearrange(
                        "kc p hd -> p kc hd"
                    ),
                )
                E = epool.tile([128, NKC, S], bf16, tag="E", bufs=2, name="E")
                sum_ps = {}
                o_ps = {}
                for qh in range(2):
                    sum_ps[qh] = pspool.tile(
                        [1, 512], fp32, tag=f"b{2 + qh}", name=f"sum{qh}"
                    )
                    o_ps[qh] = pspool.tile(
                        [128, 512], fp32, tag=f"b{4 + qh}", name=f"o{qh}"
                    )
                for kc in range(NKC):
                    bt = stream.tile([128, S], bf16, tag="bt", name="bt")
                    nc.sync.dma_start(bt[:], biasT[kc * 128 : (kc + 1) * 128, :])
                    for qh in range(2):
                        qs = slice(qh * 512, (qh + 1) * 512)
                        stag = (0, 1, 6, 7)[(kc * 2 + qh) % 4]
                        sps = pspool.tile(
                            [128, 512], fp32, tag=f"b{stag}", name="sps"
                        )
                        nc.tensor.matmul(
                            sps[:],
                            kh_t[:, kc * 128 : (kc + 1) * 128],
                            qh_t[:, qs],
                            start=True,
                            stop=True,
                        )
                        nc.vector.tensor_add(E[:, kc, qs], sps[:], bt[:, qs])
                        nc.scalar.activation(
                            E[:, kc, qs], E[:, kc, qs], Act.Exp
                        )
                        # sum and o both consume E chunk-by-chunk, so the
                        # PE never waits for the whole row of E
                        nc.tensor.matmul(
                            sum_ps[qh][:],
                            ones_sb[:],
                            E[:, kc, qs],
                            start=(kc == 0),
                            stop=(kc == NKC - 1),
                        )
                        nc.tensor.matmul(
                            o_ps[qh][:],
                            vh_t[:, kc, :],
                            E[:, kc, qs],
                            start=(kc == 0),
                            stop=(kc == NKC - 1),
                        )
                inv_row = tmppool.tile(
                    [1, S], fp32, tag="inv", bufs=2, name="inv_row"
                )
                inv_bc = tmppool.tile(
                    [128, S], fp32, tag="invbc", bufs=1, name="inv_bc"
                )
                for qh in range(2):
                    nc.vector.reciprocal_approx_fast(
                        inv_row[:, qh * 512 : (qh + 1) * 512], sum_ps[qh][:]
                    )
                for qh in range(2):
                    qs = slice(qh * 512, (qh + 1) * 512)
                    # rank-1 PE broadcast: ones[128] x inv_row -> [128, 512]
                    bc_ps = pspool.tile(
                        [128, 512], fp32, tag=f"b{2 + qh}", name="bc_ps"
                    )
                    nc.tensor.matmul(
                        bc_ps[:],
                        ones_row_sb[:],
                        inv_row[:, qs],
                        start=True,
                        stop=True,
                    )
                    nc.vector.tensor_copy(inv_bc[:, qs], bc_ps[:])
                    nc.vector.tensor_mul(
                        oT_sb[:, h, qs], o_ps[qh][:], inv_bc[:, qs]
                    )

            # ========= Phase 3: out-projection + chunked ReduceScatter =========
            # chunked by (m-quarter, token-half): wo read twice total; the
            # wo_bias/2 is folded in as a rank-1 accumulation matmul so PSUM
            # evacuation is a single ACT copy (fast bank release); each
            # [512,1024] bf16 chunk ReduceScatters over the pair while the
            # next chunk computes.
            for mh4 in range(4):
                for tg in range(2):
                    rs_in = dram.tile(
                        [512, 1024], bf16, tag="rsin", bufs=2, name="rs_in"
                    )
                    rs_out = dram.tile(
                        [256, 1024], bf16, tag="rsout", bufs=2, name="rs_out"
                    )
                    ps = {}
                    for ti in range(4):
                        for mc2 in range(2):
                            ps[(ti, mc2)] = pspool.tile(
                                [128, 512], fp32,
                                tag=f"b{ti * 2 + mc2}",
                                name=f"pso{ti}{mc2}",
                            )
                    for cc in range(HG):
                        wt = wvpool.tile(
                            [128, 1024], bf16, tag="wo", bufs=4, name="wot"
                        )
                        nc.sync.dma_start(
                            wt[:],
                            wo[cc * 128 : (cc + 1) * 128,
                               mh4 * 1024 : (mh4 + 1) * 1024],
                        )
                        for ti in range(4):
                            t128 = tg * 4 + ti
                            for mc2 in range(2):
                                nc.tensor.matmul(
                                    ps[(ti, mc2)][:],
                                    oT_sb[:, cc, t128 * 128 : (t128 + 1) * 128],
                                    wt[:, mc2 * 512 : (mc2 + 1) * 512],
                                    start=(cc == 0),
                                    stop=False,
                                )
                    for ti in range(4):
                        for mc2 in range(2):
                            m0 = mh4 * 1024 + mc2 * 512
                            nc.tensor.matmul(
                                ps[(ti, mc2)][:],
                                ones_row_bf_sb[:],
                                bo_sb[0:1, m0 : m0 + 512],
                                start=False,
                                stop=True,
                            )
                    for ti in range(4):
                        t128 = tg * 4 + ti
                        po = outpool.tile([128, 1024], bf16, tag="po", name="po")
                        for mc2 in range(2):
                            nc.scalar.activation(
                                po[:, mc2 * 512 : (mc2 + 1) * 512],
                                ps[(ti, mc2)][:],
                                Act.Copy,
                            )
                        nc.gpsimd.dma_start(
                            rs_in[ti * 128 : (ti + 1) * 128, :], po[:]
                        )
                    nc.gpsimd.collective_compute(
                        "ReduceScatter",
                        mybir.AluOpType.add,
                        replica_groups=RG,
                        ins=[rs_in[:].opt()],
                        outs=[rs_out[:].opt()],
                    )
                    for dh in range(2):
                        fin_bf = evacpool.tile(
                            [128, 1024], bf16, tag="finbf", name="fin_bf"
                        )
                        fin_f32 = evacpool.tile(
                            [128, 1024], fp32, tag="finf32", name="fin_f32"
                        )
                        nc.scalar.dma_start(
                            fin_bf[:], rs_out[dh * 128 : (dh + 1) * 128, :]
                        )
                        nc.vector.tensor_copy(fin_f32[:], fin_bf[:])
                        nc.scalar.dma_start(out[mh4, tg, dh, :, :], fin_f32[:])

    nc.finalize()
    return nc


def _prep_shards(x, attn_bias, wq_kernel, wq_bias, wk_kernel, wk_bias,
                 wv_kernel, wv_bias, wo_kernel, wo_bias):
    """Host-side shard prep. Returns in_maps (list of 8 dicts)."""
    freqs = 1.0 / 10000.0 ** (np.arange(0, ROTARY, 2) / ROTARY)  # [16]
    pos = np.arange(MAX_POS - S, MAX_POS)  # [S]
    ang = np.outer(freqs, pos)  # [16, S]
    rotC = np.cos(ang).astype(np.float32)
    rotS = np.sin(ang).astype(np.float32)
    ones = np.ones((128, 1), dtype=BF16)
    biasT = np.ascontiguousarray(attn_bias[0, 0].T).astype(BF16)

    in_maps = []
    for c in range(NCORES):
        b, g = c // 2, c % 2
        hs = slice(g * HG, (g + 1) * HG)
        in_maps.append(
            {
                "xT": np.ascontiguousarray(x[b].T).astype(BF16),
                "wq": (wq_kernel[:, hs, :].reshape(D, F) * SCALE).astype(BF16),
                "wk": wk_kernel[:, hs, :].reshape(D, F).astype(BF16),
                "wv": wv_kernel[:, hs, :].reshape(D, F).astype(BF16),
                "wo": wo_kernel[hs].reshape(F, D).astype(BF16),
                "bqT": np.ascontiguousarray((wq_bias[hs] * SCALE).T).astype(
                    np.float32
                ),
                "bkT": np.ascontiguousarray(wk_bias[hs].T).astype(np.float32),
                "bv_bc": np.broadcast_to(
                    wv_bias[hs].reshape(1, F), (128, F)
                ).astype(BF16).copy(),
                "bo_bc": np.broadcast_to(
                    (wo_bias * 0.5).reshape(1, D), (128, D)
                ).astype(BF16).copy(),
                "biasT": biasT,
                "rotC": rotC.astype(BF16),
                "rotS": rotS.astype(BF16),
                "ones": ones,
                "ones_row": np.ones((1, 128), dtype=np.float32),
                "ones_row_bf": np.ones((1, 128), dtype=BF16),
            }
        )
    return in_maps


def kernel(x, attn_bias, wq_kernel, wq_bias, wk_kernel, wk_bias,
           wv_kernel, wv_bias, wo_kernel, wo_bias, _trace=False):
    from concourse import bass_utils

    if "nc" not in _cache:
        _cache["nc"] = _build()
    nc = _cache["nc"]

    in_maps = _prep_shards(
        np.asarray(x), np.asarray(attn_bias),
        np.asarray(wq_kernel), np.asarray(wq_bias),
        np.asarray(wk_kernel), np.asarray(wk_bias),
        np.asarray(wv_kernel), np.asarray(wv_bias),
        np.asarray(wo_kernel), np.asarray(wo_bias),
    )
    res = bass_utils.run_bass_kernel_spmd(
        nc, in_maps, core_ids=list(range(NCORES)), trace=_trace
    )
    _cache["last_results"] = res

    full = np.empty((B, S, D), dtype=np.float32)
    for b in range(B):
        lo = res.results[2 * b]["out"]  # [mh4, tg, 2, 128, 1024]
        hi = res.results[2 * b + 1]["out"]
        for mh4 in range(4):
            ms = slice(mh4 * 1024, (mh4 + 1) * 1024)
            for tg in range(2):
                t0 = tg * 512
                full[b, t0 : t0 + 256, ms] = lo[mh4, tg].reshape(256, 1024)
                full[b, t0 + 256 : t0 + 512, ms] = hi[mh4, tg].reshape(256, 1024)
    return full
